# revision 50
# baseline (speedup 1.0000x reference)
"""Causal self-attention on 8 TRN2 NeuronCores.

Sharding: core c handles batch b = c//2 and head-group g = c%2 (8 of 16 heads).
Each core computes its partial y^T = w_proj[slice].T @ o^T (contraction over its
512 o-channels); the host sums the two partials per batch and adds b_proj.

Shapes (hardcoded): B=4, T=2048, C=1024, H=16, HD=64.

All matmul operands are bf16 (x/w_qkv/w_proj cast on host); accumulation is
fp32 in PSUM. x^T is loaded straight from DRAM with the xbar transpose DMA
(issues split across the SP and ACT queues; weight DMAs issued first).
o stays in SBUF (bf16) and feeds proj directly.

Schedule: attention is ACT(exp)-bound, so qkv/v/proj work is emitted in
half-unit (4-matmul) chunks interleaved between attention chunks, keeping the
PE stream dense while ACT crunches exp without starving its 2-deep score
backlog. proj for window m runs inside pair 3 right after (3, m) completes.
Diagonal causal masking is a DVE multiply with a tril mask (gpsimd
affine_select is broken for bf16 on HW, and gpsimd cannot read PSUM).

PSUM (8 banks): ps_main 2x[128,1024] holds score tiles AND filler accumulators
(split filler halves interleave 1:1 with score allocs so rotation deps always
point backward); ps_pv 2x[128,1024] holds the per-window PV accumulator — both
heads side by side, so one reciprocal-normalize chain covers the window.
reciprocal_approx_fast needs its input at partition offset 0 (HW bug), hence
the denominator row is first copied to a [1,1024] sbuf tile.
"""

import numpy as np

B, T, C, H = 4, 2048, 1024, 16
HD = C // H          # 64
G = 2                # head groups
NHL = H // G         # 8 heads per core
GQ = NHL * HD        # 512 channel slice per core
P = 128
NT = T // P          # 16 token tiles / k-chunks
NCHUNK = C // P      # 8 contraction chunks for qkv
SCALE = 1.0 / float(np.sqrt(HD))

_PROGRAM = None


def _emit(ctx, tc, aps, mybir, bass):
    nc = tc.nc
    f32 = mybir.dt.float32
    bf16 = mybir.dt.bfloat16
    EXP = mybir.ActivationFunctionType.Exp

    xa_d, xb_d, wqk_d, wv_d, bqk_d, bv_d, wp_d, yT_d = (
        aps["xa"], aps["xb"], aps["wqk"], aps["wv"], aps["bqk"], aps["bv"],
        aps["wp"], aps["yT"],
    )

    # ---------------- pools ----------------
    const = ctx.enter_context(tc.tile_pool(name="const", bufs=1))
    ps_main = ctx.enter_context(tc.tile_pool(name="ps_main", bufs=2, space="PSUM"))
    ps_pv = ctx.enter_context(tc.tile_pool(name="ps_pv", bufs=1, space="PSUM"))
    ps_fill = ctx.enter_context(tc.tile_pool(name="ps_fill", bufs=2, space="PSUM"))

    qkp = ctx.enter_context(tc.tile_pool(name="qkp", bufs=8))
    vap = ctx.enter_context(tc.tile_pool(name="vap", bufs=16))
    ptp = ctx.enter_context(tc.tile_pool(name="ptp", bufs=3))
    otp = ctx.enter_context(tc.tile_pool(name="otp", bufs=16))
    rcp = ctx.enter_context(tc.tile_pool(name="rcp", bufs=2))
    xTp = ctx.enter_context(tc.tile_pool(name="xTp", bufs=1))
    wqkp = ctx.enter_context(tc.tile_pool(name="wqkp", bufs=4))
    wvp = ctx.enter_context(tc.tile_pool(name="wvp", bufs=1))
    wpp = ctx.enter_context(tc.tile_pool(name="wpp", bufs=1))
    ysp = ctx.enter_context(tc.tile_pool(name="ysp", bufs=8))

    # constants (bias DMAs issued early on the scalar queue)
    bqk_sb = const.tile([P, 8], f32)
    bvb = const.tile([P, GQ], f32)
    ones8 = const.tile([P, NHL, 1], f32)
    nc.vector.memset(ones8[:], 1.0)
    # warm-up operand (zeros) + f32 ones row for the tail PE-broadcast
    wz = const.tile([P, P], bf16)
    nc.vector.memset(wz[:], 0.0)
    onesf = const.tile([1, HD], f32)
    nc.vector.memset(onesf[:], 1.0)
    # tril causal mask, bf16: keep pt[p, j] where j >= p (q_local >= k_local)
    trilf = const.tile([P, P], f32)
    nc.vector.memset(trilf[:], 1.0)
    nc.gpsimd.affine_select(
        out=trilf[:], in_=trilf[:], compare_op=mybir.AluOpType.is_ge,
        fill=0.0, base=0, pattern=[[1, P]], channel_multiplier=-1)
    trilb = const.tile([P, P], bf16)
    nc.vector.tensor_copy(trilb[:], trilf[:])

    # ---------------- weight DMAs (host pre-arranged: contiguous rows) ---
    wqk_tiles = {}

    def load_wqk(ct, eng=None):
        w_t = wqkp.tile([P, NCHUNK, P], bf16, name=f"wqk_{ct}", tag="wqk")
        (eng or nc.scalar).dma_start(w_t[:], wqk_d[ct])
        wqk_tiles[ct] = w_t

    # startup weights ride the ACT hwdge queue (idle until the first exp,
    # and plain 2D DMAs are safe there — only the transpose DMA corrupts)
    # so the serialized transpose stream below starts immediately.
    load_wqk(0, nc.scalar)

    # ---------------- xT via plain DMA (pre-transposed on host) ----------
    # One [128, r, t] tile so each load batch is a SINGLE queue instruction
    # (each DMA instruction costs ~650ns of queue serialization). x arrives
    # from the host already transposed and [p, r, t]-ordered, so loads are
    # plain contiguous-source DMAs (no device transpose, which is
    # packet-rate capped at ~100 GB/s).
    xT_ = xTp.tile([P, NCHUNK, T], bf16, name="xT", tag="xT")
    xT = [xT_[:, r, :] for r in range(NCHUNK)]
    # DMA priority: the prework waits only on wqk(0,4) + xa, split across
    # BOTH queues; everything else queues behind so it cannot steal HBM
    # bandwidth from the critical prefix. bv arrives as a 2KB row and is
    # broadcast on-chip by gpsimd.
    nc.sync.dma_start(xT_[:, 0:4, 0:512], xa_d[:, 0:4, :])
    nc.scalar.dma_start(xT_[:, 4:NCHUNK, 0:512], xa_d[:, 4:NCHUNK, :])
    load_wqk(4, nc.scalar)
    nc.scalar.dma_start(bqk_sb[:], bqk_d[:])
    bvs = const.tile([1, GQ], f32)
    nc.scalar.dma_start(bvs[:], bv_d[None, :])
    nc.gpsimd.partition_broadcast(bvb[:], bvs[:])
    wv_t = wvp.tile([P, NCHUNK, GQ], bf16, name="wv", tag="wv")
    nc.scalar.dma_start(wv_t[:], wv_d[:])
    nc.sync.dma_start(xT_[:, :, 512:1024], xb_d[:, :, 0:512])
    nc.sync.dma_start(xT_[:, :, 1024:T], xb_d[:, :, 512:1536])
    wp_t = wpp.tile([P, 4, C], bf16, name="wp", tag="wp")
    nc.sync.dma_start(wp_t[:], wp_d[:])

    # ---------------- qkv / proj emit units ----------------
    qkT = []  # bf16 tiles [128 c', 2048 t]; 0..3 = qT, 4..7 = kT
    for ct in range(8):
        o_t = qkp.tile([P, T], bf16, name=f"qkT{ct}", tag="qkT")
        qkT.append(o_t)

    vaug = []  # [128 k, 8 heads, 65] bf16 per k-chunk (col 64 = ones)
    for t in range(NT):
        va = vap.tile([P, NHL, HD + 1], bf16, name=f"vaug{t}", tag="vaug")
        nc.vector.tensor_copy(va[:, :, HD:HD + 1], ones8[:])
        vaug.append(va)

    def QK(ct, q, pieces=2):
        # one 512-wide quarter of qkT[ct], split into `pieces` chunks of the
        # 8-deep contraction; fillers own ps_fill so placement is free.
        st = {}
        step = NCHUNK // pieces

        def mk(pi):
            a0, a1 = pi * step, (pi + 1) * step

            def fn():
                if pi == 0:
                    if ct not in wqk_tiles:
                        load_wqk(ct)
                    st["ps"] = ps_fill.tile(
                        [P, 512], f32, name=f"qkps_{ct}_{q}", tag="fill")
                ps = st["ps"]
                for a in range(a0, a1):
                    nc.tensor.matmul(
                        ps[:], wqk_tiles[ct][:, a, :],
                        xT[a][:, q * 512:(q + 1) * 512],
                        start=(a == 0), stop=(a == NCHUNK - 1))
                if a1 == NCHUNK:
                    nc.vector.tensor_scalar_add(
                        qkT[ct][:, q * 512:(q + 1) * 512], ps[:],
                        bqk_sb[:, ct:ct + 1])
            return fn
        return [mk(pi) for pi in range(pieces)]

    def V(t, pieces=2):
        st = {}
        step = NCHUNK // pieces

        def mk(pi):
            a0, a1 = pi * step, (pi + 1) * step

            def fn():
                if pi == 0:
                    st["ps"] = ps_fill.tile(
                        [P, 512], f32, name=f"vps_{t}", tag="fill")
                ps = st["ps"]
                for a in range(a0, a1):
                    nc.tensor.matmul(
                        ps[:], xT[a][:, t * P:(t + 1) * P], wv_t[:, a, :],
                        start=(a == 0), stop=(a == NCHUNK - 1))
                if a1 == NCHUNK:
                    nc.vector.tensor_add(
                        vaug[t][:, :, 0:HD],
                        ps[:].rearrange("p (h d) -> p h d", h=NHL),
                        bvb[:].rearrange("p (h d) -> p h d", h=NHL))
            return fn
        return [mk(pi) for pi in range(pieces)]

    ot_all = {}  # (hp, m) -> [128, 512] bf16 tile in SBUF

    def cast_ys(ys, src, eng):
        if eng == "s":
            nc.scalar.activation(ys[:], src,
                                 mybir.ActivationFunctionType.Copy)
        else:
            nc.vector.tensor_copy(ys[:], src)

    def PJ(m, mt, eng="v"):
        # one cout tile (128 rows of yT) for t window m; atomic (4 matmuls)
        def fn():
            ps = ps_fill.tile([P, 512], f32, name=f"yps_{m}_{mt}", tag="fill")
            for a in range(4):
                nc.tensor.matmul(
                    ps[:], wp_t[:, a, mt * P:(mt + 1) * P],
                    ot_all[(a, m)][:, :],
                    start=(a == 0), stop=(a == 3))
            ys = ysp.tile([P, 512], bf16, name=f"ys_{m}_{mt}", tag="ys")
            cast_ys(ys, ps[:], eng)
            nc.sync.dma_start(
                yT_d[mt * P:(mt + 1) * P, m * 512:(m + 1) * 512], ys[:])
        return fn

    def PJ3_tail():
        # Window-3 proj runs entirely after the last attention chunk, when
        # the PSUM banks drain: 6 persistent accumulators take the a=0..2
        # contributions (ready since pair 2) while the (3,3) normalize chain
        # runs on DVE/ACT, keeping the PE hot; the deferred broadcast (a PE
        # matmul against a ones row, into the just-freed ps_pv bank) and ot
        # muls complete mid-stream, so the a=3 finishes follow immediately.
        ps8 = {}
        for k in range(2):
            t_ = ps_main.tile([P, 1024], f32, name=f"pjm_m{k}", tag="main")
            ps8[2 * k] = t_[:, 0:512]
            ps8[2 * k + 1] = t_[:, 512:1024]
        for k in range(2):
            t_ = ps_fill.tile([P, 512], f32, name=f"pjm_f{k}", tag="fill")
            ps8[4 + k] = t_[:]
        for mt in range(6):
            nc.tensor.matmul(
                ps8[mt], wp_t[:, 0, mt * P:(mt + 1) * P],
                ot_all[(0, 3)][:, :], start=True, stop=False)
        # deferred (3,3) normalize: broadcast 1/den via PE into the freed
        # ps_pv bank, then scale the numerators on DVE
        rc, pvs = norm33["rc"], norm33["pvs"]
        rcb_ps = ps_pv.tile([P, 1024], f32, name="rcb33", tag="ps_pv")
        for hh in range(2):
            nc.tensor.matmul(
                rcb_ps[0:HD, hh * 512:(hh + 1) * 512], onesf[:],
                rc[:, hh * 512:(hh + 1) * 512], start=True, stop=True)
        for a in (1, 2):
            for mt in range(6):
                nc.tensor.matmul(
                    ps8[mt], wp_t[:, a, mt * P:(mt + 1) * P],
                    ot_all[(a, 3)][:, :], start=False, stop=False)
        ot = otp.tile([P, 512], bf16, name="ot_3_3", tag="ot", bufs=16)
        for hh in range(2):
            nc.vector.tensor_mul(
                ot[hh * HD:(hh + 1) * HD, :],
                pvs[:, hh * 512:(hh + 1) * 512],
                rcb_ps[0:HD, hh * 512:(hh + 1) * 512])
        ot_all[(3, 3)] = ot
        for mt in range(6):
            nc.tensor.matmul(
                ps8[mt], wp_t[:, 3, mt * P:(mt + 1) * P],
                ot_all[(3, 3)][:, :], start=False, stop=True)
            ys = ysp.tile([P, 512], bf16, name=f"ys_3_{mt}", tag="ys")
            cast_ys(ys, ps8[mt], "s" if mt % 2 else "v")
            # alternate DMA queues so the tail drains at 2x
            (nc.scalar if mt % 2 else nc.sync).dma_start(
                yT_d[mt * P:(mt + 1) * P, 3 * 512:4 * 512], ys[:])
        # mt 6,7 as full 4-matmul units into the bank freed by the ot muls
        t_ = ps_pv.tile([P, 1024], f32, name="pj67", tag="ps_pv")
        for j, mt in enumerate((6, 7)):
            ps = t_[:, j * 512:(j + 1) * 512]
            for a in range(4):
                nc.tensor.matmul(
                    ps, wp_t[:, a, mt * P:(mt + 1) * P],
                    ot_all[(a, 3)][:, :], start=(a == 0), stop=(a == 3))
            ys = ysp.tile([P, 512], bf16, name=f"ys_3_{mt}", tag="ys")
            cast_ys(ys, ps, "s" if j else "v")
            (nc.scalar if j else nc.sync).dma_start(
                yT_d[mt * P:(mt + 1) * P, 3 * 512:4 * 512], ys[:])

    # ---------------- attention ----------------
    # Head pairs: head A on PE row strip 0, head B on strip 64; score pieces
    # for the two heads live in the two banks of one [128,1024] psum tile, so
    # the row-packed matmuls run concurrently and one exp covers both heads.
    # The PV accumulator is likewise one [128,1024] tile: head A cols 0:512,
    # head B cols 512:1024, partition 64 = denominators (ones column of vaug).
    norm33 = {}  # stash for the deferred (3,3) normalize: rc + pvs tiles

    def attn_pair(hp, sched, defer_last_norm=False):
        qt = qkT[hp]
        kt = qkT[4 + hp]
        for m in range(4):  # quarter windows of 512 q
            ws = m * 512
            pvt = ps_pv.tile([P, 1024], f32, name=f"pv_{hp}_{m}", tag="ps_pv")
            for i in range(4 * m + 4):  # causal k-chunks for this window
                s = max(i * P, ws)
                o = s - ws
                # head A piece in cols [o, 512), head B in [512, 1024-o)
                sc = ps_main.tile([P, 1024], f32, name=f"sc_{hp}_{m}_{i}",
                                  tag="main")
                for hh in range(2):
                    r0 = hh * HD
                    c0 = o if hh == 0 else 512
                    nc.tensor.matmul(
                        sc[:, c0:c0 + 512 - o],
                        kt[r0:r0 + HD, i * P:(i + 1) * P],
                        qt[r0:r0 + HD, s:ws + 512],
                        start=True,
                        stop=True,
                    )
                pt = ptp.tile([P, 1024], bf16, name=f"pt_{hp}_{m}_{i}",
                              tag="pt")
                diag = i * P >= ws
                if diag:
                    # split the exp per head so head A's tril mul + PV can
                    # start while head B's exp is still running
                    nc.scalar.activation(pt[:, o:512], sc[:, o:512],
                                         EXP, scale=SCALE)
                    nc.scalar.activation(pt[:, 512:1024 - o],
                                         sc[:, 512:1024 - o],
                                         EXP, scale=SCALE)
                else:
                    nc.scalar.activation(pt[:, o:1024 - o], sc[:, o:1024 - o],
                                         EXP, scale=SCALE)
                # filler between the exp issue and the exp-dependent PV
                # matmuls: the in-order PE works through it while ACT
                # computes the exp, instead of stalling at PV.
                for fn in sched.get((m, i), ()):
                    fn()
                for hh in range(2):
                    c0 = o if hh == 0 else 512
                    if diag:
                        nc.vector.tensor_mul(
                            pt[:, c0:c0 + P], pt[:, c0:c0 + P], trilb[:])
                    nc.tensor.matmul(
                        pvt[0:HD + 1, hh * 512 + o:(hh + 1) * 512],
                        vaug[i][:, 2 * hp + hh, :],
                        pt[:, c0:c0 + 512 - o],
                        start=(i == 0),
                        stop=(i == 4 * m + 3),
                    )
            if defer_last_norm and m == 3:
                # (3,3): evacuate fast (dn+recip on DVE, numerators on the
                # now-idle ACT); the broadcast + ot muls are emitted later by
                # PJ3_tail so the PE queue isn't blocked behind the recip.
                dn = rcp.tile([1, 1024], f32, name=f"dn_{hp}_{m}", tag="dn")
                nc.vector.tensor_copy(dn[:], pvt[HD:HD + 1, :])
                rc = rcp.tile([1, 1024], f32, name=f"rc_{hp}_{m}", tag="rc")
                nc.vector.reciprocal_approx_fast(rc[:], dn[:])
                pvs = rcp.tile([HD, 1024], f32, name=f"pvs_{hp}_{m}",
                               tag="pvs")
                nc.scalar.activation(pvs[:], pvt[0:HD, :],
                                     mybir.ActivationFunctionType.Copy)
                norm33["rc"] = rc
                norm33["pvs"] = pvs
                continue
            # normalize both heads at once: denominators to sbuf partition 0
            # (reciprocal_approx_fast mishandles nonzero partition offsets).
            # pvt is single-buffered, so evacuate it fast: the denominators
            # ride DVE while the numerators ride the (locally idle) ACT, so
            # pvt frees after ~1.2us instead of a 2.4us serial DVE chain.
            dn = rcp.tile([1, 1024], f32, name=f"dn_{hp}_{m}", tag="dn")
            nc.vector.tensor_copy(dn[:], pvt[HD:HD + 1, :])
            rc = rcp.tile([1, 1024], f32, name=f"rc_{hp}_{m}", tag="rc")
            nc.vector.reciprocal_approx_fast(rc[:], dn[:])
            pvs = rcp.tile([HD, 1024], f32, name=f"pvs_{hp}_{m}", tag="pvs")
            nc.scalar.activation(pvs[:], pvt[0:HD, :],
                                 mybir.ActivationFunctionType.Copy)
            rcb = rcp.tile([HD, 1024], f32, name=f"rcb_{hp}_{m}", tag="rcb")
            nc.gpsimd.partition_broadcast(rcb[:], rc[:])
            ot = otp.tile([P, 512], bf16, name=f"ot_{hp}_{m}", tag="ot",
                          bufs=16)
            for hh in range(2):
                nc.vector.tensor_mul(
                    ot[hh * HD:(hh + 1) * HD, :],
                    pvs[:, hh * 512:(hh + 1) * 512],
                    rcb[:, hh * 512:(hh + 1) * 512])
            ot_all[(hp, m)] = ot

    # ---------------- schedule ----------------
    # PE warm-up: throwaway matmuls starting as soon as the framework
    # preamble ends (~6us), so the HAM clock gate reaches 8/8 (needs ~3.4us
    # of sustained activity) by the time the real prework arrives — which
    # otherwise runs at the cold 1.2 GHz clock.
    warm_ps = ps_main.tile([P, 1024], f32, name="warm", tag="main")
    for k in range(56):
        nc.tensor.matmul(warm_ps[:, (k % 8) * P:(k % 8 + 1) * P],
                         wz[:], wz[:], start=True, stop=True)

    # pre-work: ONLY the two qk quarters window 0 needs — everything else
    # (including V(0..3)) rides the window-0 filler slots so the first
    # scores/exps are not queued behind work that waits on later DMAs.
    for ct in (0, 4):
        for f in QK(ct, 0):
            f()

    def mk():
        return {}

    def put(s, m, i, unit):
        # place unit pieces at consecutive chunks starting at (m, i)
        for k, f in enumerate(unit):
            s.setdefault((m, i + k), []).append(f)

    # pair 0: carries all remaining V units + its own q/k quarters + pair 1
    # q0/k0 — packed, so 2-piece units
    s0 = mk()
    put(s0, 0, 0, V(0, 1))    # whole unit inside slot 0, before PV(0,0,0)
    put(s0, 0, 1, V(1, 1))
    put(s0, 0, 2, V(2, 1))
    put(s0, 0, 3, V(3, 1))
    put(s0, 0, 2, QK(0, 1))   # due w1c0
    put(s0, 1, 0, QK(4, 1))   # due w1c4
    put(s0, 1, 0, V(4))       # due w1c4
    put(s0, 1, 2, V(5))       # due w1c5
    put(s0, 1, 3, V(6))       # due w1c6
    put(s0, 1, 5, V(7))       # due w1c7
    put(s0, 1, 6, QK(0, 2))   # due w2c0
    put(s0, 2, 0, QK(4, 2))   # due w2c8
    put(s0, 2, 2, V(8))
    put(s0, 2, 4, V(9))
    put(s0, 2, 6, V(10))
    put(s0, 2, 8, V(11))
    put(s0, 2, 10, QK(0, 3))  # due w3c0
    put(s0, 3, 0, QK(4, 3))   # due w3c12
    put(s0, 3, 2, V(12))
    put(s0, 3, 4, V(13))
    put(s0, 3, 6, V(14))
    put(s0, 3, 8, V(15))
    put(s0, 3, 10, QK(1, 0))  # pair 1 w0
    put(s0, 3, 12, QK(5, 0))

    # pairs 1-3 have slack, but window-start slots need >= ~1.1us of filler
    # (one exp latency) or the first PV of the window exposes a PE bubble —
    # so use 2-piece units (~850ns/slot) rather than 4-piece
    s1 = mk()
    put(s1, 0, 0, QK(1, 1))          # due p1w1c0; 2-piece (w0 is short)
    put(s1, 1, 0, QK(5, 1))          # due w1c4
    put(s1, 1, 4, QK(1, 2))          # due w2c0
    put(s1, 2, 0, QK(5, 2))          # due w2c8
    put(s1, 2, 4, QK(1, 3))
    put(s1, 2, 8, QK(2, 0))
    put(s1, 3, 0, QK(5, 3))          # due w3c12
    put(s1, 3, 4, QK(6, 0))
    put(s1, 3, 8, QK(2, 1))          # due p2w1c0

    s2 = mk()
    put(s2, 1, 0, QK(6, 1))
    put(s2, 1, 4, QK(2, 2))
    put(s2, 2, 0, QK(6, 2))
    put(s2, 2, 4, QK(2, 3))
    put(s2, 2, 8, QK(3, 0))
    put(s2, 3, 0, QK(6, 3))
    put(s2, 3, 4, QK(7, 0))
    put(s2, 3, 8, QK(3, 1))

    # pair-3 placements respect the ~5us normalize-chain latency: PJ(m, .)
    # needs ot(3, m), which lands one chain after window m's last PV, so
    # PJ(0) waits until w2 and PJ(m) never leads its chain.
    s3 = mk()
    put(s3, 1, 0, QK(7, 1))          # due w1c4
    put(s3, 1, 4, QK(3, 2))          # due w2c0
    put(s3, 2, 0, QK(7, 2))          # due w2c8
    put(s3, 2, 4, QK(3, 3))          # due w3c0
    put(s3, 2, 4, [PJ(0, k) for k in range(8)])
    put(s3, 3, 0, QK(7, 3))          # due w3c12
    put(s3, 3, 0, [PJ(1, k) for k in range(8)])
    put(s3, 3, 8, [PJ(2, k) for k in range(6)])
    # slots 14/15 stay filler-free so the final PVs (and with them the last
    # normalize chain) fire as early as possible.

    attn_pair(0, s0)
    attn_pair(1, s1)
    attn_pair(2, s2)
    attn_pair(3, s3, defer_last_norm=True)
    # PJ(2, 6..7) after the last attention chunk (independent of window 3,
    # casts on the now-idle ACT), then the window-3 proj tail overlapping
    # the deferred normalize chain.
    PJ(2, 6, "s")()
    PJ(2, 7, "s")()
    PJ3_tail()


def _build_program():
    import contextlib

    import concourse.bass as bass
    import concourse.mybir as mybir
    import concourse.tile as tile
    from concourse import bacc

    nc = bacc.Bacc("TRN2", target_bir_lowering=False, debug=False, num_devices=8)
    f32 = mybir.dt.float32
    bf16 = mybir.dt.bfloat16
    aps = {
        # x pre-transposed on host: x[r, p, t] = x_orig[t, r*128+p], split
        # into the t 0:512 prefix (xa, unblocks prework fast) and the rest
        # (xb) so each DMA reads a fully contiguous block
        "xa": nc.dram_tensor("xa", [P, NCHUNK, 512], bf16,
                             kind="ExternalInput").ap(),
        "xb": nc.dram_tensor("xb", [P, NCHUNK, T - 512], bf16,
                             kind="ExternalInput").ap(),
        # weights pre-arranged on host for contiguous per-partition loads:
        # wqk[ct, p, a*128+j] = w_qkv[a*128+p, (q|k slice) ct*128+j]
        "wqk": nc.dram_tensor("wqk", [8, P, NCHUNK * P], bf16,
                              kind="ExternalInput").ap(),
        # wv[p, a, j] = w_qkv[a*128+p, v-slice j]
        "wv": nc.dram_tensor("wv", [P, NCHUNK, GQ], bf16,
                             kind="ExternalInput").ap(),
        "bqk": nc.dram_tensor("bqk", [P, 8], f32, kind="ExternalInput").ap(),
        "bv": nc.dram_tensor("bv", [GQ], f32, kind="ExternalInput").ap(),
        # wp[p, a, j] = w_proj[a*128+p (in gq slice), j]
        "wp": nc.dram_tensor("wp", [P, 4, C], bf16, kind="ExternalInput").ap(),
        "yT": nc.dram_tensor("yT", [C, T], bf16, kind="ExternalOutput").ap(),
    }
    with tile.TileContext(nc) as tc:
        with contextlib.ExitStack() as ctx:
            _emit(ctx, tc, aps, mybir, bass)
    nc.compile()
    return nc


def get_program():
    global _PROGRAM
    if _PROGRAM is None:
        _PROGRAM = _build_program()
    return _PROGRAM


def make_in_maps(x, w_qkv, b_qkv, w_proj):
    import ml_dtypes

    bf16 = ml_dtypes.bfloat16
    x = np.asarray(x, np.float32)
    w_qkv = np.asarray(w_qkv, np.float32)
    b_qkv = np.asarray(b_qkv, np.float32)
    w_proj = np.asarray(w_proj, np.float32)
    in_maps = []
    for c in range(8):
        b = c // 2
        g = c % 2
        q0 = g * GQ
        wq = w_qkv[:, q0:q0 + GQ]
        wk = w_qkv[:, C + q0:C + q0 + GQ]
        wv = w_qkv[:, 2 * C + q0:2 * C + q0 + GQ]
        # wqk[ct, p, a*128+j] = qk[a*128+p, ct*128+j] where qk = [wq | wk]
        qk = np.concatenate([wq, wk], axis=1)        # [C, 1024]
        wqk = qk.reshape(NCHUNK, P, 8, P).transpose(2, 1, 0, 3).reshape(
            8, P, NCHUNK * P)
        # wv_r[p, a, j] = wv[a*128+p, j]
        wv_r = wv.reshape(NCHUNK, P, GQ).transpose(1, 0, 2)
        # wp_r[p, a, j] = w_proj[q0 + a*128+p, j]
        wp_r = w_proj[q0:q0 + GQ, :].reshape(4, P, C).transpose(1, 0, 2)
        bq = b_qkv[q0:q0 + GQ]
        bk = b_qkv[C + q0:C + q0 + GQ]
        bqk = np.ascontiguousarray(np.concatenate([bq, bk]).reshape(8, P).T)
        bv = np.ascontiguousarray(b_qkv[2 * C + q0:2 * C + q0 + GQ])
        xp = x[b].astype(bf16).T.reshape(NCHUNK, P, T).transpose(1, 0, 2)
        in_maps.append({
            "xa": np.ascontiguousarray(xp[:, :, 0:512]),
            "xb": np.ascontiguousarray(xp[:, :, 512:]),
            "wqk": np.ascontiguousarray(wqk.astype(bf16)),
            "wv": np.ascontiguousarray(wv_r.astype(bf16)),
            "bqk": bqk,
            "bv": bv,
            "wp": np.ascontiguousarray(wp_r.astype(bf16)),
        })
    return in_maps


def combine_outputs(outs, b_proj):
    b_proj = np.asarray(b_proj, np.float32)
    y = np.empty((B, T, C), np.float32)
    for b in range(B):
        acc = (outs[2 * b].astype(np.float32)
               + outs[2 * b + 1].astype(np.float32))  # [C, T]
        y[b] = acc.T + b_proj
    return y


def kernel(x, w_qkv, b_qkv, w_proj, b_proj, _trace=False):
    from concourse import bass_utils

    nc = get_program()
    in_maps = make_in_maps(x, w_qkv, b_qkv, w_proj)
    res = bass_utils.run_bass_kernel_spmd(
        nc, in_maps, core_ids=list(range(8)), trace=_trace
    )
    outs = [r["yT"] for r in res.results]
    y = combine_outputs(outs, b_proj)
    if _trace:
        return y, res
    return y



# revision 51
# speedup vs baseline: 1.0044x; 1.0044x over previous
"""Causal self-attention on 8 TRN2 NeuronCores.

Sharding: core c handles batch b = c//2 and head-group g = c%2 (8 of 16 heads).
Each core computes its partial y^T = w_proj[slice].T @ o^T (contraction over its
512 o-channels); the host sums the two partials per batch and adds b_proj.

Shapes (hardcoded): B=4, T=2048, C=1024, H=16, HD=64.

All matmul operands are bf16 (x/w_qkv/w_proj cast on host); accumulation is
fp32 in PSUM. x^T is loaded straight from DRAM with the xbar transpose DMA
(issues split across the SP and ACT queues; weight DMAs issued first).
o stays in SBUF (bf16) and feeds proj directly.

Schedule: attention is ACT(exp)-bound, so qkv/v/proj work is emitted in
half-unit (4-matmul) chunks interleaved between attention chunks, keeping the
PE stream dense while ACT crunches exp without starving its 2-deep score
backlog. proj for window m runs inside pair 3 right after (3, m) completes.
Diagonal causal masking is a DVE multiply with a tril mask (gpsimd
affine_select is broken for bf16 on HW, and gpsimd cannot read PSUM).

PSUM (8 banks): ps_main 2x[128,1024] holds score tiles AND filler accumulators
(split filler halves interleave 1:1 with score allocs so rotation deps always
point backward); ps_pv 2x[128,1024] holds the per-window PV accumulator — both
heads side by side, so one reciprocal-normalize chain covers the window.
reciprocal_approx_fast needs its input at partition offset 0 (HW bug), hence
the denominator row is first copied to a [1,1024] sbuf tile.
"""

import numpy as np

B, T, C, H = 4, 2048, 1024, 16
HD = C // H          # 64
G = 2                # head groups
NHL = H // G         # 8 heads per core
GQ = NHL * HD        # 512 channel slice per core
P = 128
NT = T // P          # 16 token tiles / k-chunks
NCHUNK = C // P      # 8 contraction chunks for qkv
SCALE = 1.0 / float(np.sqrt(HD))

_PROGRAM = None


def _emit(ctx, tc, aps, mybir, bass):
    nc = tc.nc
    f32 = mybir.dt.float32
    bf16 = mybir.dt.bfloat16
    EXP = mybir.ActivationFunctionType.Exp

    xa_d, xb_d, wqk_d, wv_d, bqk_d, bv_d, wp_d, yT_d = (
        aps["xa"], aps["xb"], aps["wqk"], aps["wv"], aps["bqk"], aps["bv"],
        aps["wp"], aps["yT"],
    )

    # ---------------- pools ----------------
    const = ctx.enter_context(tc.tile_pool(name="const", bufs=1))
    ps_main = ctx.enter_context(tc.tile_pool(name="ps_main", bufs=2, space="PSUM"))
    ps_pv = ctx.enter_context(tc.tile_pool(name="ps_pv", bufs=1, space="PSUM"))
    ps_fill = ctx.enter_context(tc.tile_pool(name="ps_fill", bufs=2, space="PSUM"))

    qkp = ctx.enter_context(tc.tile_pool(name="qkp", bufs=8))
    vap = ctx.enter_context(tc.tile_pool(name="vap", bufs=16))
    ptp = ctx.enter_context(tc.tile_pool(name="ptp", bufs=3))
    otp = ctx.enter_context(tc.tile_pool(name="otp", bufs=16))
    rcp = ctx.enter_context(tc.tile_pool(name="rcp", bufs=2))
    xTp = ctx.enter_context(tc.tile_pool(name="xTp", bufs=1))
    wqkp = ctx.enter_context(tc.tile_pool(name="wqkp", bufs=4))
    wvp = ctx.enter_context(tc.tile_pool(name="wvp", bufs=1))
    wpp = ctx.enter_context(tc.tile_pool(name="wpp", bufs=1))
    ysp = ctx.enter_context(tc.tile_pool(name="ysp", bufs=8))

    # constants (bias DMAs issued early on the scalar queue)
    bqk_sb = const.tile([P, 8], f32)
    bvb = const.tile([P, GQ], f32)
    ones8 = const.tile([P, NHL, 1], f32)
    nc.vector.memset(ones8[:], 1.0)
    # warm-up operand (zeros) + f32 ones row for the tail PE-broadcast
    wz = const.tile([P, P], bf16)
    nc.vector.memset(wz[:], 0.0)
    onesf = const.tile([1, HD], f32)
    nc.vector.memset(onesf[:], 1.0)
    # tril causal mask, bf16: keep pt[p, j] where j >= p (q_local >= k_local)
    trilf = const.tile([P, P], f32)
    nc.vector.memset(trilf[:], 1.0)
    nc.gpsimd.affine_select(
        out=trilf[:], in_=trilf[:], compare_op=mybir.AluOpType.is_ge,
        fill=0.0, base=0, pattern=[[1, P]], channel_multiplier=-1)
    trilb = const.tile([P, P], bf16)
    nc.vector.tensor_copy(trilb[:], trilf[:])

    # ---------------- weight DMAs (host pre-arranged: contiguous rows) ---
    wqk_tiles = {}

    def load_wqk(ct, eng=None):
        w_t = wqkp.tile([P, NCHUNK, P], bf16, name=f"wqk_{ct}", tag="wqk")
        (eng or nc.scalar).dma_start(w_t[:], wqk_d[ct])
        wqk_tiles[ct] = w_t

    # startup weights ride the ACT hwdge queue (idle until the first exp,
    # and plain 2D DMAs are safe there — only the transpose DMA corrupts)
    # so the serialized transpose stream below starts immediately.
    load_wqk(0, nc.scalar)

    # ---------------- xT via plain DMA (pre-transposed on host) ----------
    # One [128, r, t] tile so each load batch is a SINGLE queue instruction
    # (each DMA instruction costs ~650ns of queue serialization). x arrives
    # from the host already transposed and [p, r, t]-ordered, so loads are
    # plain contiguous-source DMAs (no device transpose, which is
    # packet-rate capped at ~100 GB/s).
    xT_ = xTp.tile([P, NCHUNK, T], bf16, name="xT", tag="xT")
    xT = [xT_[:, r, :] for r in range(NCHUNK)]
    # DMA priority: the prework waits only on wqk(0,4) + xa, split across
    # BOTH queues; everything else queues behind so it cannot steal HBM
    # bandwidth from the critical prefix. bv arrives as a 2KB row and is
    # broadcast on-chip by gpsimd.
    nc.sync.dma_start(xT_[:, 0:4, 0:512], xa_d[:, 0:4, :])
    nc.scalar.dma_start(xT_[:, 4:NCHUNK, 0:512], xa_d[:, 4:NCHUNK, :])
    load_wqk(4, nc.scalar)
    nc.scalar.dma_start(bqk_sb[:], bqk_d[:])
    bvs = const.tile([1, GQ], f32)
    nc.scalar.dma_start(bvs[:], bv_d[None, :])
    nc.gpsimd.partition_broadcast(bvb[:], bvs[:])
    wv_t = wvp.tile([P, NCHUNK, GQ], bf16, name="wv", tag="wv")
    nc.scalar.dma_start(wv_t[:], wv_d[:])
    nc.sync.dma_start(xT_[:, :, 512:1024], xb_d[:, :, 0:512])
    nc.sync.dma_start(xT_[:, :, 1024:T], xb_d[:, :, 512:1536])
    wp_t = wpp.tile([P, 4, C], bf16, name="wp", tag="wp")
    nc.sync.dma_start(wp_t[:], wp_d[:])

    # ---------------- qkv / proj emit units ----------------
    qkT = []  # bf16 tiles [128 c', 2048 t]; 0..3 = qT, 4..7 = kT
    for ct in range(8):
        o_t = qkp.tile([P, T], bf16, name=f"qkT{ct}", tag="qkT")
        qkT.append(o_t)

    vaug = []  # [128 k, 8 heads, 65] bf16 per k-chunk (col 64 = ones)
    for t in range(NT):
        va = vap.tile([P, NHL, HD + 1], bf16, name=f"vaug{t}", tag="vaug")
        nc.vector.tensor_copy(va[:, :, HD:HD + 1], ones8[:])
        vaug.append(va)

    def QK(ct, q, pieces=2):
        # one 512-wide quarter of qkT[ct], split into `pieces` chunks of the
        # 8-deep contraction; fillers own ps_fill so placement is free.
        st = {}
        step = NCHUNK // pieces

        def mk(pi):
            a0, a1 = pi * step, (pi + 1) * step

            def fn():
                if pi == 0:
                    if ct not in wqk_tiles:
                        load_wqk(ct)
                    st["ps"] = ps_fill.tile(
                        [P, 512], f32, name=f"qkps_{ct}_{q}", tag="fill")
                ps = st["ps"]
                for a in range(a0, a1):
                    nc.tensor.matmul(
                        ps[:], wqk_tiles[ct][:, a, :],
                        xT[a][:, q * 512:(q + 1) * 512],
                        start=(a == 0), stop=(a == NCHUNK - 1))
                if a1 == NCHUNK:
                    nc.vector.tensor_scalar_add(
                        qkT[ct][:, q * 512:(q + 1) * 512], ps[:],
                        bqk_sb[:, ct:ct + 1])
            return fn
        return [mk(pi) for pi in range(pieces)]

    def V(t, pieces=2):
        st = {}
        step = NCHUNK // pieces

        def mk(pi):
            a0, a1 = pi * step, (pi + 1) * step

            def fn():
                if pi == 0:
                    st["ps"] = ps_fill.tile(
                        [P, 512], f32, name=f"vps_{t}", tag="fill")
                ps = st["ps"]
                for a in range(a0, a1):
                    nc.tensor.matmul(
                        ps[:], xT[a][:, t * P:(t + 1) * P], wv_t[:, a, :],
                        start=(a == 0), stop=(a == NCHUNK - 1))
                if a1 == NCHUNK:
                    nc.vector.tensor_add(
                        vaug[t][:, :, 0:HD],
                        ps[:].rearrange("p (h d) -> p h d", h=NHL),
                        bvb[:].rearrange("p (h d) -> p h d", h=NHL))
            return fn
        return [mk(pi) for pi in range(pieces)]

    ot_all = {}  # (hp, m) -> [128, 512] bf16 tile in SBUF

    def cast_ys(ys, src, eng):
        if eng == "s":
            nc.scalar.activation(ys[:], src,
                                 mybir.ActivationFunctionType.Copy)
        else:
            nc.vector.tensor_copy(ys[:], src)

    def PJ(m, mt, eng="v"):
        # one cout tile (128 rows of yT) for t window m; atomic (4 matmuls)
        def fn():
            ps = ps_fill.tile([P, 512], f32, name=f"yps_{m}_{mt}", tag="fill")
            for a in range(4):
                nc.tensor.matmul(
                    ps[:], wp_t[:, a, mt * P:(mt + 1) * P],
                    ot_all[(a, m)][:, :],
                    start=(a == 0), stop=(a == 3))
            ys = ysp.tile([P, 512], bf16, name=f"ys_{m}_{mt}", tag="ys")
            cast_ys(ys, ps[:], eng)
            nc.sync.dma_start(
                yT_d[mt * P:(mt + 1) * P, m * 512:(m + 1) * 512], ys[:])
        return fn

    def PJ3_tail():
        # Window-3 proj runs entirely after the last attention chunk, when
        # the PSUM banks drain: 6 persistent accumulators take the a=0..2
        # contributions (ready since pair 2) while the (3,3) normalize chain
        # runs on DVE/ACT, keeping the PE hot; the deferred broadcast (a PE
        # matmul against a ones row, into the just-freed ps_pv bank) and ot
        # muls complete mid-stream, so the a=3 finishes follow immediately.
        ps8 = {}
        for k in range(2):
            t_ = ps_main.tile([P, 1024], f32, name=f"pjm_m{k}", tag="main")
            ps8[2 * k] = t_[:, 0:512]
            ps8[2 * k + 1] = t_[:, 512:1024]
        for k in range(2):
            t_ = ps_fill.tile([P, 512], f32, name=f"pjm_f{k}", tag="fill")
            ps8[4 + k] = t_[:]
        for mt in range(6):
            nc.tensor.matmul(
                ps8[mt], wp_t[:, 0, mt * P:(mt + 1) * P],
                ot_all[(0, 3)][:, :], start=True, stop=False)
        # deferred (3,3) normalize: broadcast 1/den via PE into the freed
        # ps_pv bank, then scale the numerators on DVE
        rc, pvs = norm33["rc"], norm33["pvs"]
        rcb_ps = ps_pv.tile([P, 1024], f32, name="rcb33", tag="ps_pv")
        for hh in range(2):
            nc.tensor.matmul(
                rcb_ps[0:HD, hh * 512:(hh + 1) * 512], onesf[:],
                rc[:, hh * 512:(hh + 1) * 512], start=True, stop=True)
        for a in (1, 2):
            for mt in range(6):
                nc.tensor.matmul(
                    ps8[mt], wp_t[:, a, mt * P:(mt + 1) * P],
                    ot_all[(a, 3)][:, :], start=False, stop=False)
        ot = otp.tile([P, 512], bf16, name="ot_3_3", tag="ot", bufs=16)
        for hh in range(2):
            nc.vector.tensor_mul(
                ot[hh * HD:(hh + 1) * HD, :],
                pvs[:, hh * 512:(hh + 1) * 512],
                rcb_ps[0:HD, hh * 512:(hh + 1) * 512])
        ot_all[(3, 3)] = ot
        for mt in range(6):
            nc.tensor.matmul(
                ps8[mt], wp_t[:, 3, mt * P:(mt + 1) * P],
                ot_all[(3, 3)][:, :], start=False, stop=True)
            ys = ysp.tile([P, 512], bf16, name=f"ys_3_{mt}", tag="ys")
            cast_ys(ys, ps8[mt], "s" if mt % 2 else "v")
            # alternate DMA queues so the tail drains at 2x
            (nc.scalar if mt % 2 else nc.sync).dma_start(
                yT_d[mt * P:(mt + 1) * P, 3 * 512:4 * 512], ys[:])
        # mt 6,7 as full 4-matmul units into the bank freed by the ot muls
        t_ = ps_pv.tile([P, 1024], f32, name="pj67", tag="ps_pv")
        for j, mt in enumerate((6, 7)):
            ps = t_[:, j * 512:(j + 1) * 512]
            for a in range(4):
                nc.tensor.matmul(
                    ps, wp_t[:, a, mt * P:(mt + 1) * P],
                    ot_all[(a, 3)][:, :], start=(a == 0), stop=(a == 3))
            ys = ysp.tile([P, 512], bf16, name=f"ys_3_{mt}", tag="ys")
            cast_ys(ys, ps, "s" if j else "v")
            (nc.scalar if j else nc.sync).dma_start(
                yT_d[mt * P:(mt + 1) * P, 3 * 512:4 * 512], ys[:])

    # ---------------- attention ----------------
    # Head pairs: head A on PE row strip 0, head B on strip 64; score pieces
    # for the two heads live in the two banks of one [128,1024] psum tile, so
    # the row-packed matmuls run concurrently and one exp covers both heads.
    # The PV accumulator is likewise one [128,1024] tile: head A cols 0:512,
    # head B cols 512:1024, partition 64 = denominators (ones column of vaug).
    norm33 = {}  # stash for the deferred (3,3) normalize: rc + pvs tiles

    def attn_pair(hp, sched, defer_last_norm=False):
        qt = qkT[hp]
        kt = qkT[4 + hp]
        for m in range(4):  # quarter windows of 512 q
            ws = m * 512
            pvt = ps_pv.tile([P, 1024], f32, name=f"pv_{hp}_{m}", tag="ps_pv")
            for i in range(4 * m + 4):  # causal k-chunks for this window
                s = max(i * P, ws)
                o = s - ws
                # head A piece in cols [o, 512), head B in [512, 1024-o)
                sc = ps_main.tile([P, 1024], f32, name=f"sc_{hp}_{m}_{i}",
                                  tag="main")
                for hh in range(2):
                    r0 = hh * HD
                    c0 = o if hh == 0 else 512
                    nc.tensor.matmul(
                        sc[:, c0:c0 + 512 - o],
                        kt[r0:r0 + HD, i * P:(i + 1) * P],
                        qt[r0:r0 + HD, s:ws + 512],
                        start=True,
                        stop=True,
                    )
                pt = ptp.tile([P, 1024], bf16, name=f"pt_{hp}_{m}_{i}",
                              tag="pt")
                diag = i * P >= ws
                nc.scalar.activation(pt[:, o:1024 - o], sc[:, o:1024 - o],
                                     EXP, scale=SCALE)
                # filler between the exp issue and the exp-dependent PV
                # matmuls: the in-order PE works through it while ACT
                # computes the exp, instead of stalling at PV.
                for fn in sched.get((m, i), ()):
                    fn()
                for hh in range(2):
                    c0 = o if hh == 0 else 512
                    if diag:
                        nc.vector.tensor_mul(
                            pt[:, c0:c0 + P], pt[:, c0:c0 + P], trilb[:])
                    nc.tensor.matmul(
                        pvt[0:HD + 1, hh * 512 + o:(hh + 1) * 512],
                        vaug[i][:, 2 * hp + hh, :],
                        pt[:, c0:c0 + 512 - o],
                        start=(i == 0),
                        stop=(i == 4 * m + 3),
                    )
            if defer_last_norm and m == 3:
                # (3,3): evacuate fast (dn+recip on DVE, numerators on the
                # now-idle ACT); the broadcast + ot muls are emitted later by
                # PJ3_tail so the PE queue isn't blocked behind the recip.
                dn = rcp.tile([1, 1024], f32, name=f"dn_{hp}_{m}", tag="dn")
                nc.vector.tensor_copy(dn[:], pvt[HD:HD + 1, :])
                rc = rcp.tile([1, 1024], f32, name=f"rc_{hp}_{m}", tag="rc")
                nc.vector.reciprocal_approx_fast(rc[:], dn[:])
                pvs = rcp.tile([HD, 1024], f32, name=f"pvs_{hp}_{m}",
                               tag="pvs")
                nc.scalar.activation(pvs[:], pvt[0:HD, :],
                                     mybir.ActivationFunctionType.Copy)
                norm33["rc"] = rc
                norm33["pvs"] = pvs
                continue
            # normalize both heads at once: denominators to sbuf partition 0
            # (reciprocal_approx_fast mishandles nonzero partition offsets).
            # pvt is single-buffered, so evacuate it fast: the denominators
            # ride DVE while the numerators ride the (locally idle) ACT, so
            # pvt frees after ~1.2us instead of a 2.4us serial DVE chain.
            dn = rcp.tile([1, 1024], f32, name=f"dn_{hp}_{m}", tag="dn")
            nc.vector.tensor_copy(dn[:], pvt[HD:HD + 1, :])
            rc = rcp.tile([1, 1024], f32, name=f"rc_{hp}_{m}", tag="rc")
            nc.vector.reciprocal_approx_fast(rc[:], dn[:])
            pvs = rcp.tile([HD, 1024], f32, name=f"pvs_{hp}_{m}", tag="pvs")
            nc.scalar.activation(pvs[:], pvt[0:HD, :],
                                 mybir.ActivationFunctionType.Copy)
            rcb = rcp.tile([HD, 1024], f32, name=f"rcb_{hp}_{m}", tag="rcb")
            nc.gpsimd.partition_broadcast(rcb[:], rc[:])
            ot = otp.tile([P, 512], bf16, name=f"ot_{hp}_{m}", tag="ot",
                          bufs=16)
            for hh in range(2):
                nc.vector.tensor_mul(
                    ot[hh * HD:(hh + 1) * HD, :],
                    pvs[:, hh * 512:(hh + 1) * 512],
                    rcb[:, hh * 512:(hh + 1) * 512])
            ot_all[(hp, m)] = ot

    # ---------------- schedule ----------------
    # PE warm-up: throwaway matmuls starting as soon as the framework
    # preamble ends (~6us), so the HAM clock gate reaches 8/8 (needs ~3.4us
    # of sustained activity) by the time the real prework arrives — which
    # otherwise runs at the cold 1.2 GHz clock.
    warm_ps = ps_main.tile([P, 1024], f32, name="warm", tag="main")
    for k in range(56):
        nc.tensor.matmul(warm_ps[:, (k % 8) * P:(k % 8 + 1) * P],
                         wz[:], wz[:], start=True, stop=True)

    # pre-work: ONLY the two qk quarters window 0 needs — everything else
    # (including V(0..3)) rides the window-0 filler slots so the first
    # scores/exps are not queued behind work that waits on later DMAs.
    for ct in (0, 4):
        for f in QK(ct, 0):
            f()

    def mk():
        return {}

    def put(s, m, i, unit):
        # place unit pieces at consecutive chunks starting at (m, i)
        for k, f in enumerate(unit):
            s.setdefault((m, i + k), []).append(f)

    # pair 0: carries all remaining V units + its own q/k quarters + pair 1
    # q0/k0 — packed, so 2-piece units
    s0 = mk()
    put(s0, 0, 0, V(0, 1))    # whole unit inside slot 0, before PV(0,0,0)
    put(s0, 0, 1, V(1, 1))
    put(s0, 0, 2, V(2, 1))
    put(s0, 0, 3, V(3, 1))
    put(s0, 0, 2, QK(0, 1))   # due w1c0
    put(s0, 1, 0, QK(4, 1))   # due w1c4
    put(s0, 1, 0, V(4))       # due w1c4
    put(s0, 1, 2, V(5))       # due w1c5
    put(s0, 1, 3, V(6))       # due w1c6
    put(s0, 1, 5, V(7))       # due w1c7
    put(s0, 1, 6, QK(0, 2))   # due w2c0
    put(s0, 2, 0, QK(4, 2))   # due w2c8
    put(s0, 2, 2, V(8))
    put(s0, 2, 4, V(9))
    put(s0, 2, 6, V(10))
    put(s0, 2, 8, V(11))
    put(s0, 2, 10, QK(0, 3))  # due w3c0
    put(s0, 3, 0, QK(4, 3))   # due w3c12
    put(s0, 3, 2, V(12))
    put(s0, 3, 4, V(13))
    put(s0, 3, 6, V(14))
    put(s0, 3, 8, V(15))
    put(s0, 3, 10, QK(1, 0))  # pair 1 w0
    put(s0, 3, 12, QK(5, 0))

    # pairs 1-3 have slack, but window-start slots need >= ~1.1us of filler
    # (one exp latency) or the first PV of the window exposes a PE bubble —
    # so use 2-piece units (~850ns/slot) rather than 4-piece
    s1 = mk()
    put(s1, 0, 0, QK(1, 1))          # due p1w1c0; 2-piece (w0 is short)
    put(s1, 1, 0, QK(5, 1))          # due w1c4
    put(s1, 1, 4, QK(1, 2))          # due w2c0
    put(s1, 2, 0, QK(5, 2))          # due w2c8
    put(s1, 2, 4, QK(1, 3))
    put(s1, 2, 8, QK(2, 0))
    put(s1, 3, 0, QK(5, 3))          # due w3c12
    put(s1, 3, 4, QK(6, 0))
    put(s1, 3, 8, QK(2, 1))          # due p2w1c0

    s2 = mk()
    put(s2, 1, 0, QK(6, 1))
    put(s2, 1, 4, QK(2, 2))
    put(s2, 2, 0, QK(6, 2))
    put(s2, 2, 4, QK(2, 3))
    put(s2, 2, 8, QK(3, 0))
    put(s2, 3, 0, QK(6, 3))
    put(s2, 3, 4, QK(7, 0))
    put(s2, 3, 8, QK(3, 1))

    # pair-3 placements respect the ~5us normalize-chain latency: PJ(m, .)
    # needs ot(3, m), which lands one chain after window m's last PV, so
    # PJ(0) waits until w2 and PJ(m) never leads its chain.
    s3 = mk()
    put(s3, 1, 0, QK(7, 1))          # due w1c4
    put(s3, 1, 4, QK(3, 2))          # due w2c0
    put(s3, 2, 0, QK(7, 2))          # due w2c8
    put(s3, 2, 4, QK(3, 3))          # due w3c0
    put(s3, 2, 4, [PJ(0, k) for k in range(8)])
    put(s3, 3, 0, QK(7, 3))          # due w3c12
    put(s3, 3, 0, [PJ(1, k) for k in range(8)])
    put(s3, 3, 8, [PJ(2, k) for k in range(6)])
    # slots 14/15 stay filler-free so the final PVs (and with them the last
    # normalize chain) fire as early as possible.

    attn_pair(0, s0)
    attn_pair(1, s1)
    attn_pair(2, s2)
    attn_pair(3, s3, defer_last_norm=True)
    # PJ(2, 6..7) after the last attention chunk (independent of window 3,
    # casts on the now-idle ACT), then the window-3 proj tail overlapping
    # the deferred normalize chain.
    PJ(2, 6, "s")()
    PJ(2, 7, "s")()
    PJ3_tail()


def _build_program():
    import contextlib

    import concourse.bass as bass
    import concourse.mybir as mybir
    import concourse.tile as tile
    from concourse import bacc

    nc = bacc.Bacc("TRN2", target_bir_lowering=False, debug=False, num_devices=8)
    f32 = mybir.dt.float32
    bf16 = mybir.dt.bfloat16
    aps = {
        # x pre-transposed on host: x[r, p, t] = x_orig[t, r*128+p], split
        # into the t 0:512 prefix (xa, unblocks prework fast) and the rest
        # (xb) so each DMA reads a fully contiguous block
        "xa": nc.dram_tensor("xa", [P, NCHUNK, 512], bf16,
                             kind="ExternalInput").ap(),
        "xb": nc.dram_tensor("xb", [P, NCHUNK, T - 512], bf16,
                             kind="ExternalInput").ap(),
        # weights pre-arranged on host for contiguous per-partition loads:
        # wqk[ct, p, a*128+j] = w_qkv[a*128+p, (q|k slice) ct*128+j]
        "wqk": nc.dram_tensor("wqk", [8, P, NCHUNK * P], bf16,
                              kind="ExternalInput").ap(),
        # wv[p, a, j] = w_qkv[a*128+p, v-slice j]
        "wv": nc.dram_tensor("wv", [P, NCHUNK, GQ], bf16,
                             kind="ExternalInput").ap(),
        "bqk": nc.dram_tensor("bqk", [P, 8], f32, kind="ExternalInput").ap(),
        "bv": nc.dram_tensor("bv", [GQ], f32, kind="ExternalInput").ap(),
        # wp[p, a, j] = w_proj[a*128+p (in gq slice), j]
        "wp": nc.dram_tensor("wp", [P, 4, C], bf16, kind="ExternalInput").ap(),
        "yT": nc.dram_tensor("yT", [C, T], bf16, kind="ExternalOutput").ap(),
    }
    with tile.TileContext(nc) as tc:
        with contextlib.ExitStack() as ctx:
            _emit(ctx, tc, aps, mybir, bass)
    nc.compile()
    return nc


def get_program():
    global _PROGRAM
    if _PROGRAM is None:
        _PROGRAM = _build_program()
    return _PROGRAM


def make_in_maps(x, w_qkv, b_qkv, w_proj):
    import ml_dtypes

    bf16 = ml_dtypes.bfloat16
    x = np.asarray(x, np.float32)
    w_qkv = np.asarray(w_qkv, np.float32)
    b_qkv = np.asarray(b_qkv, np.float32)
    w_proj = np.asarray(w_proj, np.float32)
    in_maps = []
    for c in range(8):
        b = c // 2
        g = c % 2
        q0 = g * GQ
        wq = w_qkv[:, q0:q0 + GQ]
        wk = w_qkv[:, C + q0:C + q0 + GQ]
        wv = w_qkv[:, 2 * C + q0:2 * C + q0 + GQ]
        # wqk[ct, p, a*128+j] = qk[a*128+p, ct*128+j] where qk = [wq | wk]
        qk = np.concatenate([wq, wk], axis=1)        # [C, 1024]
        wqk = qk.reshape(NCHUNK, P, 8, P).transpose(2, 1, 0, 3).reshape(
            8, P, NCHUNK * P)
        # wv_r[p, a, j] = wv[a*128+p, j]
        wv_r = wv.reshape(NCHUNK, P, GQ).transpose(1, 0, 2)
        # wp_r[p, a, j] = w_proj[q0 + a*128+p, j]
        wp_r = w_proj[q0:q0 + GQ, :].reshape(4, P, C).transpose(1, 0, 2)
        bq = b_qkv[q0:q0 + GQ]
        bk = b_qkv[C + q0:C + q0 + GQ]
        bqk = np.ascontiguousarray(np.concatenate([bq, bk]).reshape(8, P).T)
        bv = np.ascontiguousarray(b_qkv[2 * C + q0:2 * C + q0 + GQ])
        xp = x[b].astype(bf16).T.reshape(NCHUNK, P, T).transpose(1, 0, 2)
        in_maps.append({
            "xa": np.ascontiguousarray(xp[:, :, 0:512]),
            "xb": np.ascontiguousarray(xp[:, :, 512:]),
            "wqk": np.ascontiguousarray(wqk.astype(bf16)),
            "wv": np.ascontiguousarray(wv_r.astype(bf16)),
            "bqk": bqk,
            "bv": bv,
            "wp": np.ascontiguousarray(wp_r.astype(bf16)),
        })
    return in_maps


def combine_outputs(outs, b_proj):
    b_proj = np.asarray(b_proj, np.float32)
    y = np.empty((B, T, C), np.float32)
    for b in range(B):
        acc = (outs[2 * b].astype(np.float32)
               + outs[2 * b + 1].astype(np.float32))  # [C, T]
        y[b] = acc.T + b_proj
    return y


def kernel(x, w_qkv, b_qkv, w_proj, b_proj, _trace=False):
    from concourse import bass_utils

    nc = get_program()
    in_maps = make_in_maps(x, w_qkv, b_qkv, w_proj)
    res = bass_utils.run_bass_kernel_spmd(
        nc, in_maps, core_ids=list(range(8)), trace=_trace
    )
    outs = [r["yT"] for r in res.results]
    y = combine_outputs(outs, b_proj)
    if _trace:
        return y, res
    return y



# revision 53
# speedup vs baseline: 1.0071x; 1.0027x over previous
"""Causal self-attention on 8 TRN2 NeuronCores.

Sharding: core c handles batch b = c//2 and head-group g = c%2 (8 of 16 heads).
Each core computes its partial y^T = w_proj[slice].T @ o^T (contraction over its
512 o-channels); the host sums the two partials per batch and adds b_proj.

Shapes (hardcoded): B=4, T=2048, C=1024, H=16, HD=64.

All matmul operands are bf16 (x/w_qkv/w_proj cast on host); accumulation is
fp32 in PSUM. x^T is loaded straight from DRAM with the xbar transpose DMA
(issues split across the SP and ACT queues; weight DMAs issued first).
o stays in SBUF (bf16) and feeds proj directly.

Schedule: attention is ACT(exp)-bound, so qkv/v/proj work is emitted in
half-unit (4-matmul) chunks interleaved between attention chunks, keeping the
PE stream dense while ACT crunches exp without starving its 2-deep score
backlog. proj for window m runs inside pair 3 right after (3, m) completes.
Diagonal causal masking is a DVE multiply with a tril mask (gpsimd
affine_select is broken for bf16 on HW, and gpsimd cannot read PSUM).

PSUM (8 banks): ps_main 2x[128,1024] holds score tiles AND filler accumulators
(split filler halves interleave 1:1 with score allocs so rotation deps always
point backward); ps_pv 2x[128,1024] holds the per-window PV accumulator — both
heads side by side, so one reciprocal-normalize chain covers the window.
reciprocal_approx_fast needs its input at partition offset 0 (HW bug), hence
the denominator row is first copied to a [1,1024] sbuf tile.
"""

import numpy as np

B, T, C, H = 4, 2048, 1024, 16
HD = C // H          # 64
G = 2                # head groups
NHL = H // G         # 8 heads per core
GQ = NHL * HD        # 512 channel slice per core
P = 128
NT = T // P          # 16 token tiles / k-chunks
NCHUNK = C // P      # 8 contraction chunks for qkv
SCALE = 1.0 / float(np.sqrt(HD))

_PROGRAM = None


def _emit(ctx, tc, aps, mybir, bass):
    nc = tc.nc
    f32 = mybir.dt.float32
    bf16 = mybir.dt.bfloat16
    EXP = mybir.ActivationFunctionType.Exp

    xa_d, xb_d, wqk_d, wv_d, bqk_d, bv_d, wp_d, yT_d = (
        aps["xa"], aps["xb"], aps["wqk"], aps["wv"], aps["bqk"], aps["bv"],
        aps["wp"], aps["yT"],
    )

    # ---------------- pools ----------------
    const = ctx.enter_context(tc.tile_pool(name="const", bufs=1))
    ps_main = ctx.enter_context(tc.tile_pool(name="ps_main", bufs=2, space="PSUM"))
    ps_pv = ctx.enter_context(tc.tile_pool(name="ps_pv", bufs=1, space="PSUM"))
    ps_fill = ctx.enter_context(tc.tile_pool(name="ps_fill", bufs=2, space="PSUM"))

    qkp = ctx.enter_context(tc.tile_pool(name="qkp", bufs=8))
    vap = ctx.enter_context(tc.tile_pool(name="vap", bufs=16))
    ptp = ctx.enter_context(tc.tile_pool(name="ptp", bufs=3))
    otp = ctx.enter_context(tc.tile_pool(name="otp", bufs=16))
    rcp = ctx.enter_context(tc.tile_pool(name="rcp", bufs=2))
    xTp = ctx.enter_context(tc.tile_pool(name="xTp", bufs=1))
    wqkp = ctx.enter_context(tc.tile_pool(name="wqkp", bufs=4))
    wvp = ctx.enter_context(tc.tile_pool(name="wvp", bufs=1))
    wpp = ctx.enter_context(tc.tile_pool(name="wpp", bufs=1))
    ysp = ctx.enter_context(tc.tile_pool(name="ysp", bufs=8))

    # constants (bias DMAs issued early on the scalar queue)
    bqk_sb = const.tile([P, 8], f32)
    bvb = const.tile([P, GQ], f32)
    ones8 = const.tile([P, NHL, 1], f32)
    nc.vector.memset(ones8[:], 1.0)
    # warm-up operand (zeros) + f32 ones row for the tail PE-broadcast
    wz = const.tile([P, P], bf16)
    nc.vector.memset(wz[:], 0.0)
    onesf = const.tile([1, HD], f32)
    nc.vector.memset(onesf[:], 1.0)
    # tril causal mask, bf16: keep pt[p, j] where j >= p (q_local >= k_local)
    trilf = const.tile([P, P], f32)
    nc.vector.memset(trilf[:], 1.0)
    nc.gpsimd.affine_select(
        out=trilf[:], in_=trilf[:], compare_op=mybir.AluOpType.is_ge,
        fill=0.0, base=0, pattern=[[1, P]], channel_multiplier=-1)
    trilb = const.tile([P, P], bf16)
    nc.vector.tensor_copy(trilb[:], trilf[:])

    # ---------------- weight DMAs (host pre-arranged: contiguous rows) ---
    wqk_tiles = {}

    def load_wqk(ct, eng=None):
        w_t = wqkp.tile([P, NCHUNK, P], bf16, name=f"wqk_{ct}", tag="wqk")
        (eng or nc.scalar).dma_start(w_t[:], wqk_d[ct])
        wqk_tiles[ct] = w_t

    # startup weights ride the ACT hwdge queue (idle until the first exp,
    # and plain 2D DMAs are safe there — only the transpose DMA corrupts)
    # so the serialized transpose stream below starts immediately.
    load_wqk(0, nc.scalar)

    # ---------------- xT via plain DMA (pre-transposed on host) ----------
    # One [128, r, t] tile so each load batch is a SINGLE queue instruction
    # (each DMA instruction costs ~650ns of queue serialization). x arrives
    # from the host already transposed and [p, r, t]-ordered, so loads are
    # plain contiguous-source DMAs (no device transpose, which is
    # packet-rate capped at ~100 GB/s).
    xT_ = xTp.tile([P, NCHUNK, T], bf16, name="xT", tag="xT")
    xT = [xT_[:, r, :] for r in range(NCHUNK)]
    # DMA priority: the prework waits only on wqk(0,4) + xa, split across
    # BOTH queues; everything else queues behind so it cannot steal HBM
    # bandwidth from the critical prefix. bv arrives as a 2KB row and is
    # broadcast on-chip by gpsimd.
    nc.sync.dma_start(xT_[:, 0:2, 0:512], xa_d[:, 0:2, :])
    nc.sync.dma_start(xT_[:, 2:4, 0:512], xa_d[:, 2:4, :])
    nc.scalar.dma_start(xT_[:, 4:NCHUNK, 0:512], xa_d[:, 4:NCHUNK, :])
    load_wqk(4, nc.sync)
    nc.scalar.dma_start(bqk_sb[:], bqk_d[:])
    bvs = const.tile([1, GQ], f32)
    nc.scalar.dma_start(bvs[:], bv_d[None, :])
    nc.gpsimd.partition_broadcast(bvb[:], bvs[:])
    wv_t = wvp.tile([P, NCHUNK, GQ], bf16, name="wv", tag="wv")
    nc.scalar.dma_start(wv_t[:], wv_d[:])
    nc.sync.dma_start(xT_[:, :, 512:1024], xb_d[:, :, 0:512])
    nc.sync.dma_start(xT_[:, :, 1024:T], xb_d[:, :, 512:1536])
    wp_t = wpp.tile([P, 4, C], bf16, name="wp", tag="wp")
    nc.sync.dma_start(wp_t[:], wp_d[:])

    # ---------------- qkv / proj emit units ----------------
    qkT = []  # bf16 tiles [128 c', 2048 t]; 0..3 = qT, 4..7 = kT
    for ct in range(8):
        o_t = qkp.tile([P, T], bf16, name=f"qkT{ct}", tag="qkT")
        qkT.append(o_t)

    vaug = []  # [128 k, 8 heads, 65] bf16 per k-chunk (col 64 = ones)
    for t in range(NT):
        va = vap.tile([P, NHL, HD + 1], bf16, name=f"vaug{t}", tag="vaug")
        nc.vector.tensor_copy(va[:, :, HD:HD + 1], ones8[:])
        vaug.append(va)

    def QK(ct, q, pieces=2):
        # one 512-wide quarter of qkT[ct], split into `pieces` chunks of the
        # 8-deep contraction; fillers own ps_fill so placement is free.
        st = {}
        step = NCHUNK // pieces

        def mk(pi):
            a0, a1 = pi * step, (pi + 1) * step

            def fn():
                if pi == 0:
                    if ct not in wqk_tiles:
                        load_wqk(ct)
                    st["ps"] = ps_fill.tile(
                        [P, 512], f32, name=f"qkps_{ct}_{q}", tag="fill")
                ps = st["ps"]
                for a in range(a0, a1):
                    nc.tensor.matmul(
                        ps[:], wqk_tiles[ct][:, a, :],
                        xT[a][:, q * 512:(q + 1) * 512],
                        start=(a == 0), stop=(a == NCHUNK - 1))
                if a1 == NCHUNK:
                    nc.vector.tensor_scalar_add(
                        qkT[ct][:, q * 512:(q + 1) * 512], ps[:],
                        bqk_sb[:, ct:ct + 1])
            return fn
        return [mk(pi) for pi in range(pieces)]

    def V(t, pieces=2):
        st = {}
        step = NCHUNK // pieces

        def mk(pi):
            a0, a1 = pi * step, (pi + 1) * step

            def fn():
                if pi == 0:
                    st["ps"] = ps_fill.tile(
                        [P, 512], f32, name=f"vps_{t}", tag="fill")
                ps = st["ps"]
                for a in range(a0, a1):
                    nc.tensor.matmul(
                        ps[:], xT[a][:, t * P:(t + 1) * P], wv_t[:, a, :],
                        start=(a == 0), stop=(a == NCHUNK - 1))
                if a1 == NCHUNK:
                    nc.vector.tensor_add(
                        vaug[t][:, :, 0:HD],
                        ps[:].rearrange("p (h d) -> p h d", h=NHL),
                        bvb[:].rearrange("p (h d) -> p h d", h=NHL))
            return fn
        return [mk(pi) for pi in range(pieces)]

    ot_all = {}  # (hp, m) -> [128, 512] bf16 tile in SBUF

    def cast_ys(ys, src, eng):
        if eng == "s":
            nc.scalar.activation(ys[:], src,
                                 mybir.ActivationFunctionType.Copy)
        else:
            nc.vector.tensor_copy(ys[:], src)

    def PJ(m, mt, eng="v"):
        # one cout tile (128 rows of yT) for t window m; atomic (4 matmuls)
        def fn():
            ps = ps_fill.tile([P, 512], f32, name=f"yps_{m}_{mt}", tag="fill")
            for a in range(4):
                nc.tensor.matmul(
                    ps[:], wp_t[:, a, mt * P:(mt + 1) * P],
                    ot_all[(a, m)][:, :],
                    start=(a == 0), stop=(a == 3))
            ys = ysp.tile([P, 512], bf16, name=f"ys_{m}_{mt}", tag="ys")
            cast_ys(ys, ps[:], eng)
            nc.sync.dma_start(
                yT_d[mt * P:(mt + 1) * P, m * 512:(m + 1) * 512], ys[:])
        return fn

    def PJ3_tail():
        # Window-3 proj runs entirely after the last attention chunk, when
        # the PSUM banks drain: 6 persistent accumulators take the a=0..2
        # contributions (ready since pair 2) while the (3,3) normalize chain
        # runs on DVE/ACT, keeping the PE hot; the deferred broadcast (a PE
        # matmul against a ones row, into the just-freed ps_pv bank) and ot
        # muls complete mid-stream, so the a=3 finishes follow immediately.
        ps8 = {}
        for k in range(2):
            t_ = ps_main.tile([P, 1024], f32, name=f"pjm_m{k}", tag="main")
            ps8[2 * k] = t_[:, 0:512]
            ps8[2 * k + 1] = t_[:, 512:1024]
        for k in range(2):
            t_ = ps_fill.tile([P, 512], f32, name=f"pjm_f{k}", tag="fill")
            ps8[4 + k] = t_[:]
        for mt in range(6):
            nc.tensor.matmul(
                ps8[mt], wp_t[:, 0, mt * P:(mt + 1) * P],
                ot_all[(0, 3)][:, :], start=True, stop=False)
        # deferred (3,3) normalize: broadcast 1/den via PE into the freed
        # ps_pv bank, then scale the numerators on DVE
        rc, pvs = norm33["rc"], norm33["pvs"]
        rcb_ps = ps_pv.tile([P, 1024], f32, name="rcb33", tag="ps_pv")
        for hh in range(2):
            nc.tensor.matmul(
                rcb_ps[0:HD, hh * 512:(hh + 1) * 512], onesf[:],
                rc[:, hh * 512:(hh + 1) * 512], start=True, stop=True)
        for a in (1, 2):
            for mt in range(6):
                nc.tensor.matmul(
                    ps8[mt], wp_t[:, a, mt * P:(mt + 1) * P],
                    ot_all[(a, 3)][:, :], start=False, stop=False)
        ot = otp.tile([P, 512], bf16, name="ot_3_3", tag="ot", bufs=16)
        for hh in range(2):
            nc.vector.tensor_mul(
                ot[hh * HD:(hh + 1) * HD, :],
                pvs[:, hh * 512:(hh + 1) * 512],
                rcb_ps[0:HD, hh * 512:(hh + 1) * 512])
        ot_all[(3, 3)] = ot
        for mt in range(6):
            nc.tensor.matmul(
                ps8[mt], wp_t[:, 3, mt * P:(mt + 1) * P],
                ot_all[(3, 3)][:, :], start=False, stop=True)
            ys = ysp.tile([P, 512], bf16, name=f"ys_3_{mt}", tag="ys")
            cast_ys(ys, ps8[mt], "s" if mt % 2 else "v")
            # alternate DMA queues so the tail drains at 2x
            (nc.scalar if mt % 2 else nc.sync).dma_start(
                yT_d[mt * P:(mt + 1) * P, 3 * 512:4 * 512], ys[:])
        # mt 6,7 as full 4-matmul units into the bank freed by the ot muls
        t_ = ps_pv.tile([P, 1024], f32, name="pj67", tag="ps_pv")
        for j, mt in enumerate((6, 7)):
            ps = t_[:, j * 512:(j + 1) * 512]
            for a in range(4):
                nc.tensor.matmul(
                    ps, wp_t[:, a, mt * P:(mt + 1) * P],
                    ot_all[(a, 3)][:, :], start=(a == 0), stop=(a == 3))
            ys = ysp.tile([P, 512], bf16, name=f"ys_3_{mt}", tag="ys")
            cast_ys(ys, ps, "s" if j else "v")
            (nc.scalar if j else nc.sync).dma_start(
                yT_d[mt * P:(mt + 1) * P, 3 * 512:4 * 512], ys[:])

    # ---------------- attention ----------------
    # Head pairs: head A on PE row strip 0, head B on strip 64; score pieces
    # for the two heads live in the two banks of one [128,1024] psum tile, so
    # the row-packed matmuls run concurrently and one exp covers both heads.
    # The PV accumulator is likewise one [128,1024] tile: head A cols 0:512,
    # head B cols 512:1024, partition 64 = denominators (ones column of vaug).
    norm33 = {}  # stash for the deferred (3,3) normalize: rc + pvs tiles

    def attn_pair(hp, sched, defer_last_norm=False):
        qt = qkT[hp]
        kt = qkT[4 + hp]
        for m in range(4):  # quarter windows of 512 q
            ws = m * 512
            pvt = ps_pv.tile([P, 1024], f32, name=f"pv_{hp}_{m}", tag="ps_pv")
            for i in range(4 * m + 4):  # causal k-chunks for this window
                s = max(i * P, ws)
                o = s - ws
                # head A piece in cols [o, 512), head B in [512, 1024-o)
                sc = ps_main.tile([P, 1024], f32, name=f"sc_{hp}_{m}_{i}",
                                  tag="main")
                for hh in range(2):
                    r0 = hh * HD
                    c0 = o if hh == 0 else 512
                    nc.tensor.matmul(
                        sc[:, c0:c0 + 512 - o],
                        kt[r0:r0 + HD, i * P:(i + 1) * P],
                        qt[r0:r0 + HD, s:ws + 512],
                        start=True,
                        stop=True,
                    )
                pt = ptp.tile([P, 1024], bf16, name=f"pt_{hp}_{m}_{i}",
                              tag="pt")
                diag = i * P >= ws
                nc.scalar.activation(pt[:, o:1024 - o], sc[:, o:1024 - o],
                                     EXP, scale=SCALE)
                # filler between the exp issue and the exp-dependent PV
                # matmuls: the in-order PE works through it while ACT
                # computes the exp, instead of stalling at PV.
                for fn in sched.get((m, i), ()):
                    fn()
                for hh in range(2):
                    c0 = o if hh == 0 else 512
                    if diag:
                        nc.vector.tensor_mul(
                            pt[:, c0:c0 + P], pt[:, c0:c0 + P], trilb[:])
                    nc.tensor.matmul(
                        pvt[0:HD + 1, hh * 512 + o:(hh + 1) * 512],
                        vaug[i][:, 2 * hp + hh, :],
                        pt[:, c0:c0 + 512 - o],
                        start=(i == 0),
                        stop=(i == 4 * m + 3),
                    )
            if defer_last_norm and m == 3:
                # (3,3): evacuate fast (dn+recip on DVE, numerators on the
                # now-idle ACT); the broadcast + ot muls are emitted later by
                # PJ3_tail so the PE queue isn't blocked behind the recip.
                dn = rcp.tile([1, 1024], f32, name=f"dn_{hp}_{m}", tag="dn")
                nc.vector.tensor_copy(dn[:], pvt[HD:HD + 1, :])
                rc = rcp.tile([1, 1024], f32, name=f"rc_{hp}_{m}", tag="rc")
                nc.vector.reciprocal_approx_fast(rc[:], dn[:])
                pvs = rcp.tile([HD, 1024], f32, name=f"pvs_{hp}_{m}",
                               tag="pvs")
                nc.scalar.activation(pvs[:], pvt[0:HD, :],
                                     mybir.ActivationFunctionType.Copy)
                norm33["rc"] = rc
                norm33["pvs"] = pvs
                continue
            # normalize both heads at once: denominators to sbuf partition 0
            # (reciprocal_approx_fast mishandles nonzero partition offsets).
            # pvt is single-buffered, so evacuate it fast: the denominators
            # ride DVE while the numerators ride the (locally idle) ACT, so
            # pvt frees after ~1.2us instead of a 2.4us serial DVE chain.
            dn = rcp.tile([1, 1024], f32, name=f"dn_{hp}_{m}", tag="dn")
            nc.vector.tensor_copy(dn[:], pvt[HD:HD + 1, :])
            rc = rcp.tile([1, 1024], f32, name=f"rc_{hp}_{m}", tag="rc")
            nc.vector.reciprocal_approx_fast(rc[:], dn[:])
            pvs = rcp.tile([HD, 1024], f32, name=f"pvs_{hp}_{m}", tag="pvs")
            nc.scalar.activation(pvs[:], pvt[0:HD, :],
                                 mybir.ActivationFunctionType.Copy)
            rcb = rcp.tile([HD, 1024], f32, name=f"rcb_{hp}_{m}", tag="rcb")
            nc.gpsimd.partition_broadcast(rcb[:], rc[:])
            ot = otp.tile([P, 512], bf16, name=f"ot_{hp}_{m}", tag="ot",
                          bufs=16)
            for hh in range(2):
                nc.vector.tensor_mul(
                    ot[hh * HD:(hh + 1) * HD, :],
                    pvs[:, hh * 512:(hh + 1) * 512],
                    rcb[:, hh * 512:(hh + 1) * 512])
            ot_all[(hp, m)] = ot

    # ---------------- schedule ----------------
    # PE warm-up: throwaway matmuls starting as soon as the framework
    # preamble ends (~6us), so the HAM clock gate reaches 8/8 (needs ~3.4us
    # of sustained activity) by the time the real prework arrives — which
    # otherwise runs at the cold 1.2 GHz clock.
    warm_ps = ps_main.tile([P, 1024], f32, name="warm", tag="main")
    for k in range(56):
        nc.tensor.matmul(warm_ps[:, (k % 8) * P:(k % 8 + 1) * P],
                         wz[:], wz[:], start=True, stop=True)

    # pre-work: ONLY the two qk quarters window 0 needs — everything else
    # (including V(0..3)) rides the window-0 filler slots so the first
    # scores/exps are not queued behind work that waits on later DMAs.
    # 4-piece units so matmuls start on the earliest DMA arrivals.
    for ct in (0, 4):
        for f in QK(ct, 0, 4):
            f()

    def mk():
        return {}

    def put(s, m, i, unit):
        # place unit pieces at consecutive chunks starting at (m, i)
        for k, f in enumerate(unit):
            s.setdefault((m, i + k), []).append(f)

    # pair 0: carries all remaining V units + its own q/k quarters + pair 1
    # q0/k0 — packed, so 2-piece units
    s0 = mk()
    put(s0, 0, 0, V(0, 1))    # whole unit inside slot 0, before PV(0,0,0)
    put(s0, 0, 1, V(1, 1))
    put(s0, 0, 2, V(2, 1))
    put(s0, 0, 3, V(3, 1))
    put(s0, 0, 2, QK(0, 1))   # due w1c0
    put(s0, 1, 0, QK(4, 1))   # due w1c4
    put(s0, 1, 0, V(4))       # due w1c4
    put(s0, 1, 2, V(5))       # due w1c5
    put(s0, 1, 3, V(6))       # due w1c6
    put(s0, 1, 5, V(7))       # due w1c7
    put(s0, 1, 6, QK(0, 2))   # due w2c0
    put(s0, 2, 0, QK(4, 2))   # due w2c8
    put(s0, 2, 2, V(8))
    put(s0, 2, 4, V(9))
    put(s0, 2, 6, V(10))
    put(s0, 2, 8, V(11))
    put(s0, 2, 10, QK(0, 3))  # due w3c0
    put(s0, 3, 0, QK(4, 3))   # due w3c12
    put(s0, 3, 2, V(12))
    put(s0, 3, 4, V(13))
    put(s0, 3, 6, V(14))
    put(s0, 3, 8, V(15))
    put(s0, 3, 10, QK(1, 0))  # pair 1 w0
    put(s0, 3, 12, QK(5, 0))

    # pairs 1-3 have slack, but window-start slots need >= ~1.1us of filler
    # (one exp latency) or the first PV of the window exposes a PE bubble —
    # so use 2-piece units (~850ns/slot) rather than 4-piece
    s1 = mk()
    put(s1, 0, 0, QK(1, 1))          # due p1w1c0; 2-piece (w0 is short)
    put(s1, 1, 0, QK(5, 1))          # due w1c4
    put(s1, 1, 4, QK(1, 2))          # due w2c0
    put(s1, 2, 0, QK(5, 2))          # due w2c8
    put(s1, 2, 4, QK(1, 3))
    put(s1, 2, 8, QK(2, 0))
    put(s1, 3, 0, QK(5, 3))          # due w3c12
    put(s1, 3, 4, QK(6, 0))
    put(s1, 3, 8, QK(2, 1))          # due p2w1c0

    s2 = mk()
    put(s2, 1, 0, QK(6, 1))
    put(s2, 1, 4, QK(2, 2))
    put(s2, 2, 0, QK(6, 2))
    put(s2, 2, 4, QK(2, 3))
    put(s2, 2, 8, QK(3, 0))
    put(s2, 3, 0, QK(6, 3))
    put(s2, 3, 4, QK(7, 0))
    put(s2, 3, 8, QK(3, 1))

    # pair-3 placements respect the ~5us normalize-chain latency: PJ(m, .)
    # needs ot(3, m), which lands one chain after window m's last PV, so
    # PJ(0) waits until w2 and PJ(m) never leads its chain.
    s3 = mk()
    put(s3, 1, 0, QK(7, 1))          # due w1c4
    put(s3, 1, 4, QK(3, 2))          # due w2c0
    put(s3, 2, 0, QK(7, 2))          # due w2c8
    put(s3, 2, 4, QK(3, 3))          # due w3c0
    put(s3, 2, 4, [PJ(0, k) for k in range(8)])
    put(s3, 3, 0, QK(7, 3))          # due w3c12
    put(s3, 3, 0, [PJ(1, k) for k in range(8)])
    put(s3, 3, 8, [PJ(2, k) for k in range(6)])
    # slots 14/15 stay filler-free so the final PVs (and with them the last
    # normalize chain) fire as early as possible.

    attn_pair(0, s0)
    attn_pair(1, s1)
    attn_pair(2, s2)
    attn_pair(3, s3, defer_last_norm=True)
    # PJ(2, 6..7) after the last attention chunk (independent of window 3,
    # casts on the now-idle ACT), then the window-3 proj tail overlapping
    # the deferred normalize chain.
    PJ(2, 6, "s")()
    PJ(2, 7, "s")()
    PJ3_tail()


def _build_program():
    import contextlib

    import concourse.bass as bass
    import concourse.mybir as mybir
    import concourse.tile as tile
    from concourse import bacc

    nc = bacc.Bacc("TRN2", target_bir_lowering=False, debug=False, num_devices=8)
    f32 = mybir.dt.float32
    bf16 = mybir.dt.bfloat16
    aps = {
        # x pre-transposed on host: x[r, p, t] = x_orig[t, r*128+p], split
        # into the t 0:512 prefix (xa, unblocks prework fast) and the rest
        # (xb) so each DMA reads a fully contiguous block
        "xa": nc.dram_tensor("xa", [P, NCHUNK, 512], bf16,
                             kind="ExternalInput").ap(),
        "xb": nc.dram_tensor("xb", [P, NCHUNK, T - 512], bf16,
                             kind="ExternalInput").ap(),
        # weights pre-arranged on host for contiguous per-partition loads:
        # wqk[ct, p, a*128+j] = w_qkv[a*128+p, (q|k slice) ct*128+j]
        "wqk": nc.dram_tensor("wqk", [8, P, NCHUNK * P], bf16,
                              kind="ExternalInput").ap(),
        # wv[p, a, j] = w_qkv[a*128+p, v-slice j]
        "wv": nc.dram_tensor("wv", [P, NCHUNK, GQ], bf16,
                             kind="ExternalInput").ap(),
        "bqk": nc.dram_tensor("bqk", [P, 8], f32, kind="ExternalInput").ap(),
        "bv": nc.dram_tensor("bv", [GQ], f32, kind="ExternalInput").ap(),
        # wp[p, a, j] = w_proj[a*128+p (in gq slice), j]
        "wp": nc.dram_tensor("wp", [P, 4, C], bf16, kind="ExternalInput").ap(),
        "yT": nc.dram_tensor("yT", [C, T], bf16, kind="ExternalOutput").ap(),
    }
    with tile.TileContext(nc) as tc:
        with contextlib.ExitStack() as ctx:
            _emit(ctx, tc, aps, mybir, bass)
    nc.compile()
    return nc


def get_program():
    global _PROGRAM
    if _PROGRAM is None:
        _PROGRAM = _build_program()
    return _PROGRAM


def make_in_maps(x, w_qkv, b_qkv, w_proj):
    import ml_dtypes

    bf16 = ml_dtypes.bfloat16
    x = np.asarray(x, np.float32)
    w_qkv = np.asarray(w_qkv, np.float32)
    b_qkv = np.asarray(b_qkv, np.float32)
    w_proj = np.asarray(w_proj, np.float32)
    in_maps = []
    for c in range(8):
        b = c // 2
        g = c % 2
        q0 = g * GQ
        wq = w_qkv[:, q0:q0 + GQ]
        wk = w_qkv[:, C + q0:C + q0 + GQ]
        wv = w_qkv[:, 2 * C + q0:2 * C + q0 + GQ]
        # wqk[ct, p, a*128+j] = qk[a*128+p, ct*128+j] where qk = [wq | wk]
        qk = np.concatenate([wq, wk], axis=1)        # [C, 1024]
        wqk = qk.reshape(NCHUNK, P, 8, P).transpose(2, 1, 0, 3).reshape(
            8, P, NCHUNK * P)
        # wv_r[p, a, j] = wv[a*128+p, j]
        wv_r = wv.reshape(NCHUNK, P, GQ).transpose(1, 0, 2)
        # wp_r[p, a, j] = w_proj[q0 + a*128+p, j]
        wp_r = w_proj[q0:q0 + GQ, :].reshape(4, P, C).transpose(1, 0, 2)
        bq = b_qkv[q0:q0 + GQ]
        bk = b_qkv[C + q0:C + q0 + GQ]
        bqk = np.ascontiguousarray(np.concatenate([bq, bk]).reshape(8, P).T)
        bv = np.ascontiguousarray(b_qkv[2 * C + q0:2 * C + q0 + GQ])
        xp = x[b].astype(bf16).T.reshape(NCHUNK, P, T).transpose(1, 0, 2)
        in_maps.append({
            "xa": np.ascontiguousarray(xp[:, :, 0:512]),
            "xb": np.ascontiguousarray(xp[:, :, 512:]),
            "wqk": np.ascontiguousarray(wqk.astype(bf16)),
            "wv": np.ascontiguousarray(wv_r.astype(bf16)),
            "bqk": bqk,
            "bv": bv,
            "wp": np.ascontiguousarray(wp_r.astype(bf16)),
        })
    return in_maps


def combine_outputs(outs, b_proj):
    b_proj = np.asarray(b_proj, np.float32)
    y = np.empty((B, T, C), np.float32)
    for b in range(B):
        acc = (outs[2 * b].astype(np.float32)
               + outs[2 * b + 1].astype(np.float32))  # [C, T]
        y[b] = acc.T + b_proj
    return y


def kernel(x, w_qkv, b_qkv, w_proj, b_proj, _trace=False):
    from concourse import bass_utils

    nc = get_program()
    in_maps = make_in_maps(x, w_qkv, b_qkv, w_proj)
    res = bass_utils.run_bass_kernel_spmd(
        nc, in_maps, core_ids=list(range(8)), trace=_trace
    )
    outs = [r["yT"] for r in res.results]
    y = combine_outputs(outs, b_proj)
    if _trace:
        return y, res
    return y



# revision 55
# speedup vs baseline: 1.0157x; 1.0085x over previous
"""Causal self-attention on 8 TRN2 NeuronCores.

Sharding: core c handles batch b = c//2 and head-group g = c%2 (8 of 16 heads).
Each core computes its partial y^T = w_proj[slice].T @ o^T (contraction over its
512 o-channels); the host sums the two partials per batch and adds b_proj.

Shapes (hardcoded): B=4, T=2048, C=1024, H=16, HD=64.

All matmul operands are bf16 (x/w_qkv/w_proj cast on host); accumulation is
fp32 in PSUM. x^T is loaded straight from DRAM with the xbar transpose DMA
(issues split across the SP and ACT queues; weight DMAs issued first).
o stays in SBUF (bf16) and feeds proj directly.

Schedule: attention is ACT(exp)-bound, so qkv/v/proj work is emitted in
half-unit (4-matmul) chunks interleaved between attention chunks, keeping the
PE stream dense while ACT crunches exp without starving its 2-deep score
backlog. proj for window m runs inside pair 3 right after (3, m) completes.
Diagonal causal masking is a DVE multiply with a tril mask (gpsimd
affine_select is broken for bf16 on HW, and gpsimd cannot read PSUM).

PSUM (8 banks): ps_main 2x[128,1024] holds score tiles AND filler accumulators
(split filler halves interleave 1:1 with score allocs so rotation deps always
point backward); ps_pv 2x[128,1024] holds the per-window PV accumulator — both
heads side by side, so one reciprocal-normalize chain covers the window.
reciprocal_approx_fast needs its input at partition offset 0 (HW bug), hence
the denominator row is first copied to a [1,1024] sbuf tile.
"""

import numpy as np

B, T, C, H = 4, 2048, 1024, 16
HD = C // H          # 64
G = 2                # head groups
NHL = H // G         # 8 heads per core
GQ = NHL * HD        # 512 channel slice per core
P = 128
NT = T // P          # 16 token tiles / k-chunks
NCHUNK = C // P      # 8 contraction chunks for qkv
SCALE = 1.0 / float(np.sqrt(HD))

_PROGRAM = None


def _emit(ctx, tc, aps, mybir, bass):
    nc = tc.nc
    f32 = mybir.dt.float32
    bf16 = mybir.dt.bfloat16
    EXP = mybir.ActivationFunctionType.Exp

    xa_d, xb_d, wqk_d, wv_d, bqk_d, bv_d, wp_d, yT_d = (
        aps["xa"], aps["xb"], aps["wqk"], aps["wv"], aps["bqk"], aps["bv"],
        aps["wp"], aps["yT"],
    )

    # ---------------- pools ----------------
    const = ctx.enter_context(tc.tile_pool(name="const", bufs=1))
    ps_main = ctx.enter_context(tc.tile_pool(name="ps_main", bufs=2, space="PSUM"))
    ps_pv = ctx.enter_context(tc.tile_pool(name="ps_pv", bufs=1, space="PSUM"))
    ps_fill = ctx.enter_context(tc.tile_pool(name="ps_fill", bufs=2, space="PSUM"))

    qkp = ctx.enter_context(tc.tile_pool(name="qkp", bufs=8))
    vap = ctx.enter_context(tc.tile_pool(name="vap", bufs=16))
    ptp = ctx.enter_context(tc.tile_pool(name="ptp", bufs=3))
    otp = ctx.enter_context(tc.tile_pool(name="otp", bufs=16))
    rcp = ctx.enter_context(tc.tile_pool(name="rcp", bufs=2))
    xTp = ctx.enter_context(tc.tile_pool(name="xTp", bufs=1))
    wqkp = ctx.enter_context(tc.tile_pool(name="wqkp", bufs=4))
    wvp = ctx.enter_context(tc.tile_pool(name="wvp", bufs=1))
    wpp = ctx.enter_context(tc.tile_pool(name="wpp", bufs=1))
    ysp = ctx.enter_context(tc.tile_pool(name="ysp", bufs=8))

    # constants (bias DMAs issued early on the scalar queue)
    bqk_sb = const.tile([P, 8], f32)
    bvb = const.tile([P, GQ], f32)
    ones8 = const.tile([P, NHL, 1], f32)
    nc.vector.memset(ones8[:], 1.0)
    # warm-up operand (zeros) + f32 ones row for the tail PE-broadcast
    wz = const.tile([P, P], bf16)
    nc.vector.memset(wz[:], 0.0)
    onesf = const.tile([1, HD], f32)
    nc.vector.memset(onesf[:], 1.0)
    # tril causal mask, bf16: keep pt[p, j] where j >= p (q_local >= k_local)
    trilf = const.tile([P, P], f32)
    nc.vector.memset(trilf[:], 1.0)
    nc.gpsimd.affine_select(
        out=trilf[:], in_=trilf[:], compare_op=mybir.AluOpType.is_ge,
        fill=0.0, base=0, pattern=[[1, P]], channel_multiplier=-1)
    trilb = const.tile([P, P], bf16)
    nc.vector.tensor_copy(trilb[:], trilf[:])

    # ---------------- weight DMAs (host pre-arranged: contiguous rows) ---
    wqk_tiles = {}

    def load_wqk(ct, eng=None):
        w_t = wqkp.tile([P, NCHUNK, P], bf16, name=f"wqk_{ct}", tag="wqk")
        (eng or nc.scalar).dma_start(w_t[:], wqk_d[ct])
        wqk_tiles[ct] = w_t

    # startup weights ride the ACT hwdge queue (idle until the first exp,
    # and plain 2D DMAs are safe there — only the transpose DMA corrupts)
    # so the serialized transpose stream below starts immediately.
    load_wqk(0, nc.scalar)

    # ---------------- xT via plain DMA (pre-transposed on host) ----------
    # One [128, r, t] tile so each load batch is a SINGLE queue instruction
    # (each DMA instruction costs ~650ns of queue serialization). x arrives
    # from the host already transposed and [p, r, t]-ordered, so loads are
    # plain contiguous-source DMAs (no device transpose, which is
    # packet-rate capped at ~100 GB/s).
    xT_ = xTp.tile([P, NCHUNK, T], bf16, name="xT", tag="xT")
    xT = [xT_[:, r, :] for r in range(NCHUNK)]
    # DMA priority: the prework waits only on wqk(0,4) + xa, split across
    # BOTH queues; everything else queues behind so it cannot steal HBM
    # bandwidth from the critical prefix. bv arrives as a 2KB row and is
    # broadcast on-chip by gpsimd.
    nc.sync.dma_start(xT_[:, 0:2, 0:512], xa_d[:, 0:2, :])
    nc.sync.dma_start(xT_[:, 2:4, 0:512], xa_d[:, 2:4, :])
    nc.scalar.dma_start(xT_[:, 4:NCHUNK, 0:512], xa_d[:, 4:NCHUNK, :])
    load_wqk(4, nc.sync)
    nc.scalar.dma_start(bqk_sb[:], bqk_d[:])
    bvs = const.tile([1, GQ], f32)
    nc.scalar.dma_start(bvs[:], bv_d[None, :])
    nc.gpsimd.partition_broadcast(bvb[:], bvs[:])
    wv_t = wvp.tile([P, NCHUNK, GQ], bf16, name="wv", tag="wv")
    nc.scalar.dma_start(wv_t[:], wv_d[:])
    nc.sync.dma_start(xT_[:, :, 512:1024], xb_d[:, :, 0:512])
    nc.sync.dma_start(xT_[:, :, 1024:T], xb_d[:, :, 512:1536])
    wp_t = wpp.tile([P, 4, C], bf16, name="wp", tag="wp")
    nc.sync.dma_start(wp_t[:], wp_d[:])

    # ---------------- qkv / proj emit units ----------------
    qkT = []  # bf16 tiles [128 c', 2048 t]; 0..3 = qT, 4..7 = kT
    for ct in range(8):
        o_t = qkp.tile([P, T], bf16, name=f"qkT{ct}", tag="qkT")
        qkT.append(o_t)

    vaug = []  # [128 k, 8 heads, 65] bf16 per k-chunk (col 64 = ones)
    for t in range(NT):
        va = vap.tile([P, NHL, HD + 1], bf16, name=f"vaug{t}", tag="vaug")
        nc.vector.tensor_copy(va[:, :, HD:HD + 1], ones8[:])
        vaug.append(va)

    def QK(ct, q, pieces=2):
        # one 512-wide quarter of qkT[ct], split into `pieces` chunks of the
        # 8-deep contraction; fillers own ps_fill so placement is free.
        st = {}
        step = NCHUNK // pieces

        def mk(pi):
            a0, a1 = pi * step, (pi + 1) * step

            def fn():
                if pi == 0:
                    if ct not in wqk_tiles:
                        load_wqk(ct)
                    st["ps"] = ps_fill.tile(
                        [P, 512], f32, name=f"qkps_{ct}_{q}", tag="fill")
                ps = st["ps"]
                for a in range(a0, a1):
                    nc.tensor.matmul(
                        ps[:], wqk_tiles[ct][:, a, :],
                        xT[a][:, q * 512:(q + 1) * 512],
                        start=(a == 0), stop=(a == NCHUNK - 1))
                if a1 == NCHUNK:
                    nc.vector.tensor_scalar_add(
                        qkT[ct][:, q * 512:(q + 1) * 512], ps[:],
                        bqk_sb[:, ct:ct + 1])
            return fn
        return [mk(pi) for pi in range(pieces)]

    def V(t, pieces=2):
        st = {}
        step = NCHUNK // pieces

        def mk(pi):
            a0, a1 = pi * step, (pi + 1) * step

            def fn():
                if pi == 0:
                    st["ps"] = ps_fill.tile(
                        [P, 512], f32, name=f"vps_{t}", tag="fill")
                ps = st["ps"]
                for a in range(a0, a1):
                    nc.tensor.matmul(
                        ps[:], xT[a][:, t * P:(t + 1) * P], wv_t[:, a, :],
                        start=(a == 0), stop=(a == NCHUNK - 1))
                if a1 == NCHUNK:
                    nc.vector.tensor_add(
                        vaug[t][:, :, 0:HD],
                        ps[:].rearrange("p (h d) -> p h d", h=NHL),
                        bvb[:].rearrange("p (h d) -> p h d", h=NHL))
            return fn
        return [mk(pi) for pi in range(pieces)]

    ot_all = {}  # (hp, m) -> [128, 512] bf16 tile in SBUF

    def cast_ys(ys, src, eng):
        if eng == "s":
            nc.scalar.activation(ys[:], src,
                                 mybir.ActivationFunctionType.Copy)
        else:
            nc.vector.tensor_copy(ys[:], src)

    def PJ(m, mt, eng="v"):
        # one cout tile (128 rows of yT) for t window m; atomic (4 matmuls)
        def fn():
            ps = ps_fill.tile([P, 512], f32, name=f"yps_{m}_{mt}", tag="fill")
            for a in range(4):
                nc.tensor.matmul(
                    ps[:], wp_t[:, a, mt * P:(mt + 1) * P],
                    ot_all[(a, m)][:, :],
                    start=(a == 0), stop=(a == 3))
            ys = ysp.tile([P, 512], bf16, name=f"ys_{m}_{mt}", tag="ys")
            cast_ys(ys, ps[:], eng)
            nc.sync.dma_start(
                yT_d[mt * P:(mt + 1) * P, m * 512:(m + 1) * 512], ys[:])
        return fn

    def PJ3_tail():
        # Window-3 proj runs entirely after the last attention chunk, when
        # the PSUM banks drain: 6 persistent accumulators take the a=0..2
        # contributions (ready since pair 2) while the (3,3) normalize chain
        # runs on DVE/ACT, keeping the PE hot; the deferred broadcast (a PE
        # matmul against a ones row, into the just-freed ps_pv bank) and ot
        # muls complete mid-stream, so the a=3 finishes follow immediately.
        ps8 = {}
        for k in range(2):
            t_ = ps_main.tile([P, 1024], f32, name=f"pjm_m{k}", tag="main")
            ps8[2 * k] = t_[:, 0:512]
            ps8[2 * k + 1] = t_[:, 512:1024]
        for k in range(2):
            t_ = ps_fill.tile([P, 512], f32, name=f"pjm_f{k}", tag="fill")
            ps8[4 + k] = t_[:]
        for mt in range(6):
            nc.tensor.matmul(
                ps8[mt], wp_t[:, 0, mt * P:(mt + 1) * P],
                ot_all[(0, 3)][:, :], start=True, stop=False)
        # deferred (3,3) normalize: broadcast 1/den via PE into the freed
        # ps_pv bank, then scale the numerators on DVE
        rc, pvs = norm33["rc"], norm33["pvs"]
        rcb_ps = ps_pv.tile([P, 1024], f32, name="rcb33", tag="ps_pv")
        for hh in range(2):
            nc.tensor.matmul(
                rcb_ps[0:HD, hh * 512:(hh + 1) * 512], onesf[:],
                rc[:, hh * 512:(hh + 1) * 512], start=True, stop=True)
        for a in (1, 2):
            for mt in range(6):
                nc.tensor.matmul(
                    ps8[mt], wp_t[:, a, mt * P:(mt + 1) * P],
                    ot_all[(a, 3)][:, :], start=False, stop=False)
        ot = otp.tile([P, 512], bf16, name="ot_3_3", tag="ot", bufs=16)
        for hh in range(2):
            nc.vector.tensor_mul(
                ot[hh * HD:(hh + 1) * HD, :],
                pvs[:, hh * 512:(hh + 1) * 512],
                rcb_ps[0:HD, hh * 512:(hh + 1) * 512])
        ot_all[(3, 3)] = ot
        for mt in range(6):
            nc.tensor.matmul(
                ps8[mt], wp_t[:, 3, mt * P:(mt + 1) * P],
                ot_all[(3, 3)][:, :], start=False, stop=True)
            ys = ysp.tile([P, 512], bf16, name=f"ys_3_{mt}", tag="ys")
            cast_ys(ys, ps8[mt], "s" if mt % 2 else "v")
            # alternate DMA queues so the tail drains at 2x
            (nc.scalar if mt % 2 else nc.sync).dma_start(
                yT_d[mt * P:(mt + 1) * P, 3 * 512:4 * 512], ys[:])
        # mt 6,7 as full 4-matmul units into the bank freed by the ot muls
        t_ = ps_pv.tile([P, 1024], f32, name="pj67", tag="ps_pv")
        for j, mt in enumerate((6, 7)):
            ps = t_[:, j * 512:(j + 1) * 512]
            for a in range(4):
                nc.tensor.matmul(
                    ps, wp_t[:, a, mt * P:(mt + 1) * P],
                    ot_all[(a, 3)][:, :], start=(a == 0), stop=(a == 3))
            ys = ysp.tile([P, 512], bf16, name=f"ys_3_{mt}", tag="ys")
            cast_ys(ys, ps, "s" if j else "v")
            (nc.scalar if j else nc.sync).dma_start(
                yT_d[mt * P:(mt + 1) * P, 3 * 512:4 * 512], ys[:])

    # ---------------- attention ----------------
    # Head pairs: head A on PE row strip 0, head B on strip 64; score pieces
    # for the two heads live in the two banks of one [128,1024] psum tile, so
    # the row-packed matmuls run concurrently and one exp covers both heads.
    # The PV accumulator is likewise one [128,1024] tile: head A cols 0:512,
    # head B cols 512:1024, partition 64 = denominators (ones column of vaug).
    norm33 = {}  # stash for the deferred (3,3) normalize: rc + pvs tiles

    def attn_pair(hp, sched, defer_last_norm=False):
        qt = qkT[hp]
        kt = qkT[4 + hp]

        def emit_scores(m, i):
            # head A piece in cols [o, 512), head B in [512, 1024-o)
            ws = m * 512
            s = max(i * P, ws)
            o = s - ws
            sc = ps_main.tile([P, 1024], f32, name=f"sc_{hp}_{m}_{i}",
                              tag="main")
            for hh in range(2):
                r0 = hh * HD
                c0 = o if hh == 0 else 512
                nc.tensor.matmul(
                    sc[:, c0:c0 + 512 - o],
                    kt[r0:r0 + HD, i * P:(i + 1) * P],
                    qt[r0:r0 + HD, s:ws + 512],
                    start=True,
                    stop=True,
                )
            return sc, o

        chunks = [(m, i) for m in range(4) for i in range(4 * m + 4)]
        pend = {chunks[0]: emit_scores(*chunks[0])}
        pvt = None
        for idx, (m, i) in enumerate(chunks):
            ws = m * 512
            if i == 0:
                pvt = ps_pv.tile([P, 1024], f32, name=f"pv_{hp}_{m}",
                                 tag="ps_pv")
            sc, o = pend.pop((m, i))
            pt = ptp.tile([P, 1024], bf16, name=f"pt_{hp}_{m}_{i}", tag="pt")
            diag = i * P >= ws
            nc.scalar.activation(pt[:, o:1024 - o], sc[:, o:1024 - o],
                                 EXP, scale=SCALE)
            # filler between the exp issue and the exp-dependent PV matmuls:
            # the in-order PE works through it while ACT computes the exp,
            # instead of stalling at PV.
            for fn in sched.get((m, i), ()):
                fn()
            # NEXT chunk's scores go ahead of this chunk's PVs in the PE
            # queue: they depend on nothing recent, so they always run while
            # ACT finishes this exp, covering the exp->PV latency even when
            # the filler slot is short.
            if idx + 1 < len(chunks):
                pend[chunks[idx + 1]] = emit_scores(*chunks[idx + 1])
            for hh in range(2):
                c0 = o if hh == 0 else 512
                if diag:
                    nc.vector.tensor_mul(
                        pt[:, c0:c0 + P], pt[:, c0:c0 + P], trilb[:])
                nc.tensor.matmul(
                    pvt[0:HD + 1, hh * 512 + o:(hh + 1) * 512],
                    vaug[i][:, 2 * hp + hh, :],
                    pt[:, c0:c0 + 512 - o],
                    start=(i == 0),
                    stop=(i == 4 * m + 3),
                )
            if i < 4 * m + 3:
                continue
            if defer_last_norm and m == 3:
                # (3,3): evacuate fast (dn+recip on DVE, numerators on the
                # now-idle ACT); the broadcast + ot muls are emitted later by
                # PJ3_tail so the PE queue isn't blocked behind the recip.
                dn = rcp.tile([1, 1024], f32, name=f"dn_{hp}_{m}", tag="dn")
                nc.vector.tensor_copy(dn[:], pvt[HD:HD + 1, :])
                rc = rcp.tile([1, 1024], f32, name=f"rc_{hp}_{m}", tag="rc")
                nc.vector.reciprocal_approx_fast(rc[:], dn[:])
                pvs = rcp.tile([HD, 1024], f32, name=f"pvs_{hp}_{m}",
                               tag="pvs")
                nc.scalar.activation(pvs[:], pvt[0:HD, :],
                                     mybir.ActivationFunctionType.Copy)
                norm33["rc"] = rc
                norm33["pvs"] = pvs
                continue
            # normalize both heads at once: denominators to sbuf partition 0
            # (reciprocal_approx_fast mishandles nonzero partition offsets).
            # pvt is single-buffered, so evacuate it fast: the denominators
            # ride DVE while the numerators ride the (locally idle) ACT, so
            # pvt frees after ~1.2us instead of a 2.4us serial DVE chain.
            dn = rcp.tile([1, 1024], f32, name=f"dn_{hp}_{m}", tag="dn")
            nc.vector.tensor_copy(dn[:], pvt[HD:HD + 1, :])
            rc = rcp.tile([1, 1024], f32, name=f"rc_{hp}_{m}", tag="rc")
            nc.vector.reciprocal_approx_fast(rc[:], dn[:])
            pvs = rcp.tile([HD, 1024], f32, name=f"pvs_{hp}_{m}", tag="pvs")
            nc.scalar.activation(pvs[:], pvt[0:HD, :],
                                 mybir.ActivationFunctionType.Copy)
            rcb = rcp.tile([HD, 1024], f32, name=f"rcb_{hp}_{m}", tag="rcb")
            nc.gpsimd.partition_broadcast(rcb[:], rc[:])
            ot = otp.tile([P, 512], bf16, name=f"ot_{hp}_{m}", tag="ot",
                          bufs=16)
            for hh in range(2):
                nc.vector.tensor_mul(
                    ot[hh * HD:(hh + 1) * HD, :],
                    pvs[:, hh * 512:(hh + 1) * 512],
                    rcb[:, hh * 512:(hh + 1) * 512])
            ot_all[(hp, m)] = ot

    # ---------------- schedule ----------------
    # PE warm-up: throwaway matmuls starting as soon as the framework
    # preamble ends (~6us), so the HAM clock gate reaches 8/8 (needs ~3.4us
    # of sustained activity) by the time the real prework arrives — which
    # otherwise runs at the cold 1.2 GHz clock.
    warm_ps = ps_main.tile([P, 1024], f32, name="warm", tag="main")
    for k in range(56):
        nc.tensor.matmul(warm_ps[:, (k % 8) * P:(k % 8 + 1) * P],
                         wz[:], wz[:], start=True, stop=True)

    # pre-work: ONLY the two qk quarters window 0 needs — everything else
    # (including V(0..3)) rides the window-0 filler slots so the first
    # scores/exps are not queued behind work that waits on later DMAs.
    # 4-piece units so matmuls start on the earliest DMA arrivals.
    for ct in (0, 4):
        for f in QK(ct, 0, 4):
            f()

    def mk():
        return {}

    def put(s, m, i, unit):
        # place unit pieces at consecutive chunks starting at (m, i)
        for k, f in enumerate(unit):
            s.setdefault((m, i + k), []).append(f)

    # pair 0: carries all remaining V units + its own q/k quarters + pair 1
    # q0/k0 — packed, so 2-piece units
    s0 = mk()
    put(s0, 0, 0, V(0, 1))    # whole unit inside slot 0, before PV(0,0,0)
    put(s0, 0, 1, V(1, 1))
    put(s0, 0, 2, V(2, 1))
    put(s0, 0, 3, V(3, 1))
    put(s0, 0, 2, QK(0, 1))   # due w1c0
    put(s0, 1, 0, QK(4, 1))   # due w1c4
    put(s0, 1, 0, V(4))       # due w1c4
    put(s0, 1, 2, V(5))       # due w1c5
    put(s0, 1, 3, V(6))       # due w1c6
    put(s0, 1, 5, V(7))       # due w1c7
    put(s0, 1, 6, QK(0, 2))   # due w2c0
    put(s0, 2, 0, QK(4, 2))   # due w2c8
    put(s0, 2, 2, V(8))
    put(s0, 2, 4, V(9))
    put(s0, 2, 6, V(10))
    put(s0, 2, 8, V(11))
    put(s0, 2, 10, QK(0, 3))  # due w3c0
    put(s0, 3, 0, QK(4, 3))   # due w3c12
    put(s0, 3, 2, V(12))
    put(s0, 3, 4, V(13))
    put(s0, 3, 6, V(14))
    put(s0, 3, 8, V(15))
    put(s0, 3, 10, QK(1, 0))  # pair 1 w0
    put(s0, 3, 12, QK(5, 0))

    # pairs 1-3 have slack, but window-start slots need >= ~1.1us of filler
    # (one exp latency) or the first PV of the window exposes a PE bubble —
    # so use 2-piece units (~850ns/slot) rather than 4-piece
    s1 = mk()
    put(s1, 0, 0, QK(1, 1))          # due p1w1c0; 2-piece (w0 is short)
    put(s1, 1, 0, QK(5, 1))          # due w1c4
    put(s1, 1, 4, QK(1, 2))          # due w2c0
    put(s1, 2, 0, QK(5, 2))          # due w2c8
    put(s1, 2, 4, QK(1, 3))
    put(s1, 2, 8, QK(2, 0))
    put(s1, 3, 0, QK(5, 3))          # due w3c12
    put(s1, 3, 4, QK(6, 0))
    put(s1, 3, 8, QK(2, 1))          # due p2w1c0

    s2 = mk()
    put(s2, 1, 0, QK(6, 1))
    put(s2, 1, 4, QK(2, 2))
    put(s2, 2, 0, QK(6, 2))
    put(s2, 2, 4, QK(2, 3))
    put(s2, 2, 8, QK(3, 0))
    put(s2, 3, 0, QK(6, 3))
    put(s2, 3, 4, QK(7, 0))
    put(s2, 3, 8, QK(3, 1))

    # pair-3 placements respect the ~5us normalize-chain latency: PJ(m, .)
    # needs ot(3, m), which lands one chain after window m's last PV, so
    # PJ(0) waits until w2 and PJ(m) never leads its chain.
    s3 = mk()
    put(s3, 1, 0, QK(7, 1))          # due w1c4
    put(s3, 1, 4, QK(3, 2))          # due w2c0
    put(s3, 2, 0, QK(7, 2))          # due w2c8
    put(s3, 2, 4, QK(3, 3))          # due w3c0
    put(s3, 2, 4, [PJ(0, k) for k in range(8)])
    put(s3, 3, 0, QK(7, 3))          # due w3c12
    put(s3, 3, 0, [PJ(1, k) for k in range(8)])
    put(s3, 3, 8, [PJ(2, k) for k in range(6)])
    # slots 14/15 stay filler-free so the final PVs (and with them the last
    # normalize chain) fire as early as possible.

    attn_pair(0, s0)
    attn_pair(1, s1)
    attn_pair(2, s2)
    attn_pair(3, s3, defer_last_norm=True)
    # PJ(2, 6..7) after the last attention chunk (independent of window 3,
    # casts on the now-idle ACT), then the window-3 proj tail overlapping
    # the deferred normalize chain.
    PJ(2, 6, "s")()
    PJ(2, 7, "s")()
    PJ3_tail()


def _build_program():
    import contextlib

    import concourse.bass as bass
    import concourse.mybir as mybir
    import concourse.tile as tile
    from concourse import bacc

    nc = bacc.Bacc("TRN2", target_bir_lowering=False, debug=False, num_devices=8)
    f32 = mybir.dt.float32
    bf16 = mybir.dt.bfloat16
    aps = {
        # x pre-transposed on host: x[r, p, t] = x_orig[t, r*128+p], split
        # into the t 0:512 prefix (xa, unblocks prework fast) and the rest
        # (xb) so each DMA reads a fully contiguous block
        "xa": nc.dram_tensor("xa", [P, NCHUNK, 512], bf16,
                             kind="ExternalInput").ap(),
        "xb": nc.dram_tensor("xb", [P, NCHUNK, T - 512], bf16,
                             kind="ExternalInput").ap(),
        # weights pre-arranged on host for contiguous per-partition loads:
        # wqk[ct, p, a*128+j] = w_qkv[a*128+p, (q|k slice) ct*128+j]
        "wqk": nc.dram_tensor("wqk", [8, P, NCHUNK * P], bf16,
                              kind="ExternalInput").ap(),
        # wv[p, a, j] = w_qkv[a*128+p, v-slice j]
        "wv": nc.dram_tensor("wv", [P, NCHUNK, GQ], bf16,
                             kind="ExternalInput").ap(),
        "bqk": nc.dram_tensor("bqk", [P, 8], f32, kind="ExternalInput").ap(),
        "bv": nc.dram_tensor("bv", [GQ], f32, kind="ExternalInput").ap(),
        # wp[p, a, j] = w_proj[a*128+p (in gq slice), j]
        "wp": nc.dram_tensor("wp", [P, 4, C], bf16, kind="ExternalInput").ap(),
        "yT": nc.dram_tensor("yT", [C, T], bf16, kind="ExternalOutput").ap(),
    }
    with tile.TileContext(nc) as tc:
        with contextlib.ExitStack() as ctx:
            _emit(ctx, tc, aps, mybir, bass)
    nc.compile()
    return nc


def get_program():
    global _PROGRAM
    if _PROGRAM is None:
        _PROGRAM = _build_program()
    return _PROGRAM


def make_in_maps(x, w_qkv, b_qkv, w_proj):
    import ml_dtypes

    bf16 = ml_dtypes.bfloat16
    x = np.asarray(x, np.float32)
    w_qkv = np.asarray(w_qkv, np.float32)
    b_qkv = np.asarray(b_qkv, np.float32)
    w_proj = np.asarray(w_proj, np.float32)
    in_maps = []
    for c in range(8):
        b = c // 2
        g = c % 2
        q0 = g * GQ
        wq = w_qkv[:, q0:q0 + GQ]
        wk = w_qkv[:, C + q0:C + q0 + GQ]
        wv = w_qkv[:, 2 * C + q0:2 * C + q0 + GQ]
        # wqk[ct, p, a*128+j] = qk[a*128+p, ct*128+j] where qk = [wq | wk]
        qk = np.concatenate([wq, wk], axis=1)        # [C, 1024]
        wqk = qk.reshape(NCHUNK, P, 8, P).transpose(2, 1, 0, 3).reshape(
            8, P, NCHUNK * P)
        # wv_r[p, a, j] = wv[a*128+p, j]
        wv_r = wv.reshape(NCHUNK, P, GQ).transpose(1, 0, 2)
        # wp_r[p, a, j] = w_proj[q0 + a*128+p, j]
        wp_r = w_proj[q0:q0 + GQ, :].reshape(4, P, C).transpose(1, 0, 2)
        bq = b_qkv[q0:q0 + GQ]
        bk = b_qkv[C + q0:C + q0 + GQ]
        bqk = np.ascontiguousarray(np.concatenate([bq, bk]).reshape(8, P).T)
        bv = np.ascontiguousarray(b_qkv[2 * C + q0:2 * C + q0 + GQ])
        xp = x[b].astype(bf16).T.reshape(NCHUNK, P, T).transpose(1, 0, 2)
        in_maps.append({
            "xa": np.ascontiguousarray(xp[:, :, 0:512]),
            "xb": np.ascontiguousarray(xp[:, :, 512:]),
            "wqk": np.ascontiguousarray(wqk.astype(bf16)),
            "wv": np.ascontiguousarray(wv_r.astype(bf16)),
            "bqk": bqk,
            "bv": bv,
            "wp": np.ascontiguousarray(wp_r.astype(bf16)),
        })
    return in_maps


def combine_outputs(outs, b_proj):
    b_proj = np.asarray(b_proj, np.float32)
    y = np.empty((B, T, C), np.float32)
    for b in range(B):
        acc = (outs[2 * b].astype(np.float32)
               + outs[2 * b + 1].astype(np.float32))  # [C, T]
        y[b] = acc.T + b_proj
    return y


def kernel(x, w_qkv, b_qkv, w_proj, b_proj, _trace=False):
    from concourse import bass_utils

    nc = get_program()
    in_maps = make_in_maps(x, w_qkv, b_qkv, w_proj)
    res = bass_utils.run_bass_kernel_spmd(
        nc, in_maps, core_ids=list(range(8)), trace=_trace
    )
    outs = [r["yT"] for r in res.results]
    y = combine_outputs(outs, b_proj)
    if _trace:
        return y, res
    return y

